# revision 32
# baseline (speedup 1.0000x reference)
"""Trainium2 Bass kernel for nn_Colar_static (retrieval_knn).

Sharding: data-parallel over batch B=2048 across 8 NeuronCores (256 rows each).
Weights/exemplars replicated, precomputed + quantized on host.

Design (vs the bf16 baseline at 53.3us):
  * Every large matmul is fp8e4m3 with the DoubleRow perf mode (K=256 per
    instruction, 0.5 cycles/row) -> 4x bf16 MAC rate and 1-byte weights
    (the kernel is DMA-bound: all DMA serializes at ~332 GB/s).
  * dots = x @ (Wk^T Ekn) directly: Wk is folded into the exemplars on the
    host, so the 2MB Wk and 0.7MB Ekn never ship; only D8 [CIN,672] (1.4MB).
  * ||k|| (softmax temperature only) via a random sketch: ||S k|| with
    S [128,1024] Gaussian, W_sk = S Wk [128, CIN] fp8 (0.25MB). The 5% norm
    error is invisible downstream (validated: rel err 3.21e-3, same as the
    exact-norm pipeline, because cos logits are tiny and softmax-smoothed).
  * v  = x8@Wv8hi + x8lo@Wv8hi + x8@Wv8lo   3-pass residual-compensated fp8
    (v dominates the output; plain fp8 fails at 3.3e-2).
  * fE = A8^T @ ut8 (fp8 DR);  out = Wout^T @ [hv;hfe] in bf16 (tiny).

Scales (all folded, no extra device work): D,W_sk x64; Wv x32; A x16; u x256.
The sketch scale cancels: rinv = rsqrt(sum((64 S k)^2)) = 1/(64||Sk||) and
dots are x64, so exp(dots*rinv) = exp(cos).

Rel err vs fp32 reference ~3.2e-3 (numpy-sim validated; gate is 2e-2).
"""

import numpy as np
import ml_dtypes

import concourse.bass as bass
import concourse.bacc as bacc
import concourse.mybir as mybir
import concourse.tile as tile
from concourse.bass_utils import run_bass_kernel_spmd

AF = mybir.ActivationFunctionType
BF = mybir.dt.bfloat16
F8 = mybir.dt.float8e4
F32 = mybir.dt.float32
DR = mybir.MatmulPerfMode.DoubleRow
bf16 = ml_dtypes.bfloat16
f8 = ml_dtypes.float8_e4m3

# Problem constants (hardcoded; kernel.py must be self-contained)
B, T, CIN, CH, M, NCLS = 2048, 8, 2048, 1024, 32, 21
NCORES = 8
BL = B // NCORES          # 256 batch rows per core
J = NCLS * M              # 672
P = 128
KB = CIN // P             # 16 contraction blocks over CIN
KP = KB // 2              # 8 DoubleRow pairs over CIN
CHB = CH // P             # 8 blocks over CH
NB = BL // P              # 2 batch chunks of 128
RSK = 128                 # norm-sketch rank
SD, SW, SA, SU = 64.0, 32.0, 16.0, 256.0
JC = [(0, 256), (256, 512), (512, J)]   # dots psum chunks (bank-safe)


def build_nc(debug=False):
    nc = bacc.Bacc("TRN2", target_bir_lowering=False, debug=debug,
                   num_devices=NCORES)

    x8_e = nc.dram_tensor("x8", [P, KB * BL], F8, kind="ExternalInput")
    x8lo_e = nc.dram_tensor("x8lo", [P, KB * BL], F8, kind="ExternalInput")
    wsk8_e = nc.dram_tensor("wsk8", [P, KB * P], F8, kind="ExternalInput")
    d8_e = nc.dram_tensor("d8", [P, KB * J], F8, kind="ExternalInput")
    wv8_e = nc.dram_tensor("wv8", [CHB, P, KB * P], F8, kind="ExternalInput")
    wv8lo_e = nc.dram_tensor("wv8lo", [CHB, P, KB * P], F8, kind="ExternalInput")
    a8_e = nc.dram_tensor("a8", [P, 6 * CH], F8, kind="ExternalInput")
    evwb_e = nc.dram_tensor("evwb", [P, J], F8, kind="ExternalInput")
    bke_e = nc.dram_tensor("bke", [1, J], BF, kind="ExternalInput")
    wout_e = nc.dram_tensor("wout", [P, KB * NCLS], BF, kind="ExternalInput")
    bsk_e = nc.dram_tensor("bsk", [P, 1], F32, kind="ExternalInput")
    bv_e = nc.dram_tensor("bv", [P, CHB], F32, kind="ExternalInput")
    bout_e = nc.dram_tensor("bout", [NCLS, 1], F32, kind="ExternalInput")
    ident_e = nc.dram_tensor("ident", [P, P], BF, kind="ExternalInput")
    out_e = nc.dram_tensor("out", [NCLS, BL], F32, kind="ExternalOutput")

    with tile.TileContext(nc) as tc:
        from contextlib import ExitStack
        with ExitStack() as ctx:
            pers = ctx.enter_context(tc.tile_pool(name="pers", bufs=1))
            pmisc = ctx.enter_context(tc.tile_pool(name="pmisc", bufs=1, space="PSUM"))
            pkv = ctx.enter_context(tc.tile_pool(name="pkv", bufs=2, space="PSUM"))
            pdot = ctx.enter_context(tc.tile_pool(name="pdot", bufs=1, space="PSUM"))
            ptr = ctx.enter_context(tc.tile_pool(name="ptr", bufs=1, space="PSUM"))
            pfe = ctx.enter_context(tc.tile_pool(name="pfe", bufs=1, space="PSUM"))

            # ---- SBUF tiles ----
            x8_s = pers.tile([P, KB, BL], F8, tag="x8")
            x8lo_s = pers.tile([P, KB, BL], F8, tag="x8lo")
            wsk8_s = pers.tile([P, KB, P], F8, tag="wsk8")
            d8_s = pers.tile([P, KB, J], F8, tag="d8")
            wv8_s = pers.tile([P, CHB, KB, P], F8, tag="wv8")
            wv8lo_s = pers.tile([P, CHB, KB, P], F8, tag="wv8lo")
            a8_s = pers.tile([P, 6, CH], F8, tag="a8")
            evwb_s = pers.tile([P, J], F8, tag="evwb")
            bke_s = pers.tile([1, J], BF, tag="bke")
            wout_s = pers.tile([P, KB, NCLS], BF, tag="wout")
            bsk_s = pers.tile([P, 1], F32, tag="bsk")
            bv_s = pers.tile([P, CHB], F32, tag="bv")
            bout_s = pers.tile([NCLS, 1], F32, tag="bout")
            ident_s = pers.tile([P, P], BF, tag="ident")
            ones_s = pers.tile([P, 1], BF, tag="ones")
            ones1_s = pers.tile([1, P], BF, tag="ones1")
            scratch_s = pers.tile([1, 1], F32, tag="scratch")
            sk_s = pers.tile([P, BL], BF, tag="sk")
            sksq_s = pers.tile([P, BL], BF, tag="sksq")
            hv_s = pers.tile([P, CHB, BL], BF, tag="hv")
            hfe_s = pers.tile([P, CHB, BL], BF, tag="hfe")
            e_s = pers.tile([P, NB, J], BF, tag="e")
            tmp_s = pers.tile([P, J], BF, tag="tmp")
            u_s = pers.tile([P, NB, J], BF, tag="u")
            ut_s = pers.tile([P, 6, BL], F8, tag="ut")
            rinv_s = pers.tile([P, NB], F32, tag="rinv")
            rs1_s = pers.tile([P, NB], F32, tag="rs1")
            rs2_s = pers.tile([P, NB], F32, tag="rs2")
            magic_s = pers.tile([P, 1], mybir.dt.int32, tag="magic")
            s_s = pers.tile([P, NB * NCLS], F32, tag="s")
            num_s = pers.tile([P, NB * NCLS], F32, tag="num")
            sinv_s = pers.tile([P, NB * NCLS], F32, tag="sinv")
            t_s = pers.tile([P, NB * NCLS], F32, tag="t")
            g_s = pers.tile([P, NB * NCLS], F32, tag="g")
            gg_s = pers.tile([P, NB], F32, tag="gg")
            ginv_s = pers.tile([P, NB], F32, tag="ginv")
            c1_s = pers.tile([P, NB * NCLS], F32, tag="c1")
            c_s = pers.tile([P, NB * NCLS], F32, tag="c")
            out_sb = pers.tile([NCLS, BL], F32, tag="outsb")

            # ---- setup: memsets + pin the Exp ACT table before any evict ----
            nc.vector.memset(ones_s[:], 1.0)
            nc.vector.memset(ones1_s[:], 1.0)
            nc.vector.memset(magic_s[:], 0x5f3759df)
            nc.vector.memset(ut_s[:], 0.0)        # zero jb-5 pad partitions
            nc.vector.memset(scratch_s[:], 1.0)
            nc.scalar.activation(scratch_s[:], scratch_s[:], AF.Exp)

            # ---- DMA schedule (sync queue; DMA device is the critical
            # resource at ~22.6us busy). wv pairs last: each gates only one
            # v pass. ----
            nc.sync.dma_start(x8_s[:], x8_e.ap())
            nc.sync.dma_start(wsk8_s[:], wsk8_e.ap())
            nc.sync.dma_start(bke_s[:], bke_e.ap())
            nc.sync.dma_start(evwb_s[:], evwb_e.ap())
            nc.sync.dma_start(d8_s[:], d8_e.ap())
            nc.sync.dma_start(x8lo_s[:], x8lo_e.ap())
            nc.sync.dma_start(wout_s[:], wout_e.ap())
            for o in range(CHB):
                nc.sync.dma_start(wv8_s[:, o, :, :], wv8_e.ap()[o])
                nc.sync.dma_start(wv8lo_s[:, o, :, :], wv8lo_e.ap()[o])
                if o == 1:
                    nc.sync.dma_start(a8_s[:], a8_e.ap())
            nc.gpsimd.dma_start(bsk_s[:], bsk_e.ap())
            nc.gpsimd.dma_start(bv_s[:], bv_e.ap())
            nc.gpsimd.dma_start(bout_s[:], bout_e.ap())
            nc.gpsimd.dma_start(ident_s[:], ident_e.ap())

            # ---- phase 1: norm sketch: sk = 64*S*k, rinv = 1/||sk|| ----
            ps = pkv.tile([P, BL], F32, tag="pkv")
            for p in range(KP):
                nc.tensor.matmul(ps[:], wsk8_s[:, 2 * p:2 * p + 2, :],
                                 x8_s[:, 2 * p:2 * p + 2, :],
                                 start=(p == 0), stop=(p == KP - 1),
                                 perf_mode=DR)
            nc.scalar.activation(sk_s[:], ps[:], AF.Identity, bias=bsk_s[:])
            nc.vector.tensor_mul(sksq_s[:], sk_s[:], sk_s[:])
            ps2 = pmisc.tile([P, NB], F32, tag="misc")
            for bc in range(NB):
                nc.tensor.matmul(ps2[:, bc:bc + 1],
                                 sksq_s[:, bc * P:(bc + 1) * P], ones_s[:],
                                 start=True, stop=True)
                sq = rs1_s[:, bc:bc + 1]
                nc.vector.tensor_copy(sq, ps2[:, bc:bc + 1])
                y = rinv_s[:, bc:bc + 1]
                nc.vector.tensor_scalar(
                    y.bitcast(mybir.dt.int32), sq.bitcast(mybir.dt.int32),
                    1, None, op0=mybir.AluOpType.logical_shift_right)
                nc.vector.tensor_tensor(
                    out=y.bitcast(mybir.dt.int32), in0=magic_s[:],
                    in1=y.bitcast(mybir.dt.int32),
                    op=mybir.AluOpType.subtract)
                for _ in range(2):
                    t1 = rs2_s[:, bc:bc + 1]
                    nc.vector.tensor_mul(t1, y, y)
                    nc.vector.tensor_mul(t1, t1, sq)
                    nc.vector.tensor_scalar(t1, t1, -0.5, 1.5,
                                            op0=mybir.AluOpType.mult,
                                            op1=mybir.AluOpType.add)
                    nc.vector.tensor_mul(y, y, t1)

            # ---- phase 2: dots = x8 @ D8 (+bkE), chunk-major fp8 DR ----
            def dots(bc):
                psd = pdot.tile([P, J], F32, tag="pdot")
                for (c0, c1) in JC:
                    for p in range(KP):
                        nc.tensor.matmul(
                            psd[:, c0:c1],
                            x8_s[:, 2 * p:2 * p + 2, bc * P:bc * P + P],
                            d8_s[:, 2 * p:2 * p + 2, c0:c1],
                            start=(p == 0), stop=False, perf_mode=DR)
                    # += bkE (K=1 rank-1 broadcast matmul closes the group)
                    nc.tensor.matmul(psd[:, c0:c1], ones1_s[:],
                                     bke_s[:, c0:c1], start=False, stop=True)
                nc.scalar.activation(e_s[:, bc, 0:512], psd[:, 0:512], AF.Exp,
                                     scale=rinv_s[:, bc:bc + 1])
                nc.scalar.activation(e_s[:, bc, 512:J], psd[:, 512:J], AF.Exp,
                                     scale=rinv_s[:, bc:bc + 1])

            def softmax_chain(bc, eng):
                # bc0 runs on DVE, bc1 on gpsimd: the two chains execute in
                # parallel so u1 lands ~2us earlier
                e_sl = e_s[:, bc, :]
                e3 = e_sl.rearrange("p (n m) -> p n m", m=M)
                ncls_sl = slice(bc * NCLS, (bc + 1) * NCLS)
                s2 = s_s[:, ncls_sl]
                eng.reduce_sum(s2, e3, axis=mybir.AxisListType.X)
                u_tmp = u_s[:, bc, :]
                eng.tensor_mul(u_tmp, e_sl, evwb_s[:])
                eng.reduce_sum(num_s[:, ncls_sl],
                               u_tmp.rearrange("p (n m) -> p n m", m=M),
                               axis=mybir.AxisListType.X)
                eng.reciprocal(sinv_s[:, ncls_sl], s2)
                eng.tensor_mul(t_s[:, ncls_sl], num_s[:, ncls_sl],
                               sinv_s[:, ncls_sl])
                nc.scalar.activation(g_s[:, ncls_sl], t_s[:, ncls_sl], AF.Exp)
                eng.reduce_sum(gg_s[:, bc:bc + 1], g_s[:, ncls_sl],
                               axis=mybir.AxisListType.X)
                eng.reciprocal(ginv_s[:, bc:bc + 1], gg_s[:, bc:bc + 1])
                # fold the u scale SU into ginv: c = g*sinv * (SU/G)
                eng.tensor_scalar(ginv_s[:, bc:bc + 1], ginv_s[:, bc:bc + 1],
                                  SU, None, op0=mybir.AluOpType.mult)
                eng.tensor_mul(c1_s[:, ncls_sl], g_s[:, ncls_sl],
                               sinv_s[:, ncls_sl])
                eng.tensor_scalar_mul(c_s[:, ncls_sl], c1_s[:, ncls_sl],
                                      ginv_s[:, bc:bc + 1])
                c_b = bass.AP(c_s.tensor, c_s[:, ncls_sl].offset,
                              c_s[:, ncls_sl].ap + [[0, M]])
                u3 = u_s[:, bc, :].rearrange("p (n m) -> p n m", m=M)
                eng.tensor_mul(u3, e3, c_b)

            # ---- out accumulator: block matmuls emitted as inputs land ----
            pso = pmisc.tile([NCLS, BL], F32, tag="misc")
            n_out_mm = [0]

            def out_mm(h_s, i):
                kb = i if h_s is hv_s else CHB + i
                nc.tensor.matmul(pso[:], wout_s[:, kb, :], h_s[:, i, :],
                                 start=(n_out_mm[0] == 0),
                                 stop=(n_out_mm[0] == KB - 1))
                n_out_mm[0] += 1

            def v_block(o):
                ps = pkv.tile([P, BL], F32, tag="pkv")
                n = 0
                for (wt, xt) in ((wv8_s, x8_s), (wv8_s, x8lo_s),
                                 (wv8lo_s, x8_s)):
                    for p in range(KP):
                        nc.tensor.matmul(ps[:], wt[:, o, 2 * p:2 * p + 2, :],
                                         xt[:, 2 * p:2 * p + 2, :],
                                         start=(n == 0), stop=(n == 3 * KP - 1),
                                         perf_mode=DR)
                        n += 1
                nc.scalar.activation(hv_s[:, o, :], ps[:], AF.Relu,
                                     scale=1.0 / SW, bias=bv_s[:, o:o + 1])

            def transpose_u(bc):
                def tgroup(grp):
                    pst = ptr.tile([P, 3 * P], BF, tag="ptr")
                    for t, jb in enumerate(grp):
                        w = P if jb < 5 else J - 5 * P
                        nc.tensor.transpose(
                            pst[:w, t * P:(t + 1) * P],
                            u_s[:, bc, jb * P:jb * P + w],
                            ident_s[:])
                    n = sum(1 for jb in grp if jb < 5)
                    base = ut_s[:, grp[0], bc * P:bc * P + P]
                    dst = bass.AP(ut_s.tensor, base.offset,
                                  [base.ap[0], [BL, n], base.ap[1]])
                    nc.vector.tensor_copy(
                        dst, pst[:, 0:n * P].rearrange("p (n q) -> p n q", q=P))
                    if n < len(grp):
                        jb = grp[n]
                        w = J - 5 * P
                        nc.scalar.activation(
                            ut_s[:w, jb, bc * P:bc * P + P],
                            pst[:w, n * P:(n + 1) * P], AF.Identity)
                tgroup((0, 1, 2))
                tgroup((3, 4, 5))

            def fe_all():
                for o in range(CHB):
                    acc = pfe.tile([P, BL], F32, tag=f"pfe{o % 2}")
                    for t in range(3):
                        nc.tensor.matmul(acc[:], a8_s[:, 2 * t:2 * t + 2,
                                                      o * P:(o + 1) * P],
                                         ut_s[:, 2 * t:2 * t + 2, :],
                                         start=(t == 0), stop=(t == 2),
                                         perf_mode=DR)
                    dst = hfe_s[:, o, :]
                    if o % 2 == 0:
                        nc.scalar.activation(dst, acc[:], AF.Relu,
                                             scale=1.0 / (SA * SU))
                    else:
                        nc.vector.tensor_scalar(dst, acc[:],
                                                1.0 / (SA * SU), 0.0,
                                                op0=mybir.AluOpType.mult,
                                                op1=mybir.AluOpType.max)

            # ---- main interleave: the whole dots/softmax/transpose/fE chain
            # runs before the wv stream thickens; v blocks then track DMA ----
            dots(0)
            softmax_chain(0, nc.vector)
            v_block(0)
            dots(1)
            softmax_chain(1, nc.vector)
            v_block(1)
            out_mm(hv_s, 0)
            transpose_u(0)
            transpose_u(1)
            fe_all()
            for i in range(CHB):
                out_mm(hfe_s, i)
            for o in range(2, CHB):
                v_block(o)
                out_mm(hv_s, o - 1)
            out_mm(hv_s, CHB - 1)

            # ---- +bout, DMA out ----
            nc.vector.tensor_scalar_add(out_sb[:], pso[:], bout_s[:, 0:1])
            nc.sync.dma_start(out_e.ap(), out_sb[:])

    nc.compile()
    return nc


def host_prep(x, static_feat, Wk, bk, Wv, bv, WEk, bEk, WEv, bEv, Ww, bw,
              Wout, bout):
    """Host-side fp32 precompute, fp8/bf16 quantization, per-core input maps."""
    EPS = 1e-8
    f32 = np.float32
    x = np.asarray(x, f32)
    static_feat = np.asarray(static_feat, f32)
    Wk, bk = np.asarray(Wk, f32), np.asarray(bk, f32)
    Wv, bv = np.asarray(Wv, f32), np.asarray(bv, f32)
    Wout, bout = np.asarray(Wout, f32), np.asarray(bout, f32)

    Ek = np.einsum('oc,ncm->nom', np.asarray(WEk, f32), static_feat,
                   optimize=True) + np.asarray(bEk, f32)[None, :, None]
    Ev = np.einsum('oc,ncm->nom', np.asarray(WEv, f32), static_feat,
                   optimize=True) + np.asarray(bEv, f32)[None, :, None]
    Ekn = Ek / np.maximum(np.linalg.norm(Ek, axis=1, keepdims=True), EPS)
    Ekn_mat = Ekn.transpose(1, 0, 2).reshape(CH, J)          # [CH, 672]
    A_mat = Ev.transpose(0, 2, 1).reshape(J, CH)             # [672, CH]
    evwb = np.einsum('nom,o->nm', Ev, np.asarray(Ww, f32)[0]).reshape(J)

    # norm sketch + folded dots
    S = np.random.RandomState(0).randn(RSK, CH).astype(f32) / np.sqrt(RSK)
    W_sk = S @ Wk                                            # [128, CIN]
    b_sk = S @ bk
    D = Wk.T @ Ekn_mat                                       # [CIN, J]
    bkE = bk @ Ekn_mat                                       # [J]

    def cinlayout(w, width):    # [CIN, width] -> [P, KB*width]
        return np.ascontiguousarray(
            w.reshape(KB, P, width).transpose(1, 0, 2).reshape(P, KB * width))

    wsk8_h = cinlayout((W_sk.T * SD).astype(f8), P)
    d8_h = cinlayout((D * SD).astype(f8), J)

    def wlayout(w):     # [CIN, OCH] f8 -> dram [OCH/P, P, KB*P]
        och = w.shape[1]
        return np.ascontiguousarray(
            w.reshape(KB, P, och // P, P).transpose(2, 1, 0, 3)
            .reshape(och // P, P, KB * P))

    wv_s = Wv.T * SW
    wv8_f = wv_s.astype(f8)
    wv8lo_f = (wv_s - wv8_f.astype(f32)).astype(f8)
    wv8_h = wlayout(wv8_f)
    wv8lo_h = wlayout(wv8lo_f)

    a_pad = np.zeros((6 * P, CH), f32)
    a_pad[:J] = A_mat * SA
    a8_h = np.ascontiguousarray(
        a_pad.astype(f8).reshape(6, P, CH).transpose(1, 0, 2).reshape(P, 6 * CH))
    evwb_h = np.ascontiguousarray(
        np.broadcast_to(evwb.astype(f8)[None, :], (P, J)))
    bke_h = (bkE * SD).astype(bf16).reshape(1, J)
    wout_h = np.ascontiguousarray(
        Wout.T.reshape(KB, P, NCLS).transpose(1, 0, 2).reshape(
            P, KB * NCLS).astype(bf16))
    bsk_h = np.ascontiguousarray((b_sk * SD).reshape(P, 1))
    bv_h = np.ascontiguousarray(bv.reshape(CHB, P).T)
    bout_h = bout.reshape(NCLS, 1)
    ident_h = np.eye(P, dtype=bf16)

    xT = np.ascontiguousarray(x[:, -1, :].T)                 # [CIN, B]
    x8_f = xT.astype(f8)
    x8lo_f = (xT - x8_f.astype(f32)).astype(f8)

    shared = dict(wsk8=wsk8_h, d8=d8_h, wv8=wv8_h, wv8lo=wv8lo_h, a8=a8_h,
                  evwb=evwb_h, bke=bke_h, wout=wout_h, bsk=bsk_h, bv=bv_h,
                  bout=bout_h, ident=ident_h)
    in_maps = []
    for c in range(NCORES):
        sl = slice(c * BL, (c + 1) * BL)

        def xlayout(xf):
            return np.ascontiguousarray(
                xf[:, sl].reshape(KB, P, BL).transpose(1, 0, 2)
                .reshape(P, KB * BL))
        in_maps.append(dict(x8=xlayout(x8_f), x8lo=xlayout(x8lo_f), **shared))
    return in_maps


_NC_CACHE = {}


def get_nc(debug=False):
    if debug not in _NC_CACHE:
        _NC_CACHE[debug] = build_nc(debug=debug)
    return _NC_CACHE[debug]


def kernel(**inputs) -> np.ndarray:
    nc = get_nc()
    in_maps = host_prep(**inputs)
    res = run_bass_kernel_spmd(nc, in_maps, list(range(NCORES)))
    out = np.empty((B, NCLS, 1), dtype=np.float32)
    for c in range(NCORES):
        out[c * BL:(c + 1) * BL, :, 0] = res.results[c]["out"].T
    return out


# revision 37
# speedup vs baseline: 1.0054x; 1.0054x over previous
"""Trainium2 Bass kernel for nn_Colar_static (retrieval_knn).

Sharding: data-parallel over batch B=2048 across 8 NeuronCores (256 rows each).
Weights/exemplars replicated, precomputed + quantized on host.

Design (vs the bf16 baseline at 53.3us):
  * Every large matmul is fp8e4m3 with the DoubleRow perf mode (K=256 per
    instruction, 0.5 cycles/row) -> 4x bf16 MAC rate and 1-byte weights
    (the kernel is DMA-bound: all DMA serializes at ~332 GB/s).
  * dots = x @ (Wk^T Ekn) directly: Wk is folded into the exemplars on the
    host, so the 2MB Wk and 0.7MB Ekn never ship; only D8 [CIN,672] (1.4MB).
  * ||k|| (softmax temperature only) via a random sketch: ||S k|| with
    S [128,1024] Gaussian, W_sk = S Wk [128, CIN] fp8 (0.25MB). The 5% norm
    error is invisible downstream (validated: rel err 3.21e-3, same as the
    exact-norm pipeline, because cos logits are tiny and softmax-smoothed).
  * v  = x8@Wv8hi + x8lo@Wv8hi + x8@Wv8lo   3-pass residual-compensated fp8
    (v dominates the output; plain fp8 fails at 3.3e-2).
  * fE = A8^T @ ut8 (fp8 DR);  out = Wout^T @ [hv;hfe] in bf16 (tiny).

Scales (all folded, no extra device work): D,W_sk x64; Wv x32; A x16; u x256.
The sketch scale cancels: rinv = rsqrt(sum((64 S k)^2)) = 1/(64||Sk||) and
dots are x64, so exp(dots*rinv) = exp(cos).

Rel err vs fp32 reference ~3.2e-3 (numpy-sim validated; gate is 2e-2).
"""

import numpy as np
import ml_dtypes

import concourse.bass as bass
import concourse.bacc as bacc
import concourse.mybir as mybir
import concourse.tile as tile
from concourse.bass_utils import run_bass_kernel_spmd

AF = mybir.ActivationFunctionType
BF = mybir.dt.bfloat16
F8 = mybir.dt.float8e4
F32 = mybir.dt.float32
DR = mybir.MatmulPerfMode.DoubleRow
bf16 = ml_dtypes.bfloat16
f8 = ml_dtypes.float8_e4m3

# Problem constants (hardcoded; kernel.py must be self-contained)
B, T, CIN, CH, M, NCLS = 2048, 8, 2048, 1024, 32, 21
NCORES = 8
BL = B // NCORES          # 256 batch rows per core
J = NCLS * M              # 672
P = 128
KB = CIN // P             # 16 contraction blocks over CIN
KP = KB // 2              # 8 DoubleRow pairs over CIN
CHB = CH // P             # 8 blocks over CH
NB = BL // P              # 2 batch chunks of 128
RSK = 64                  # norm-sketch rank
SD, SW, SA, SU = 64.0, 32.0, 16.0, 256.0
JC = [(0, 256), (256, 512), (512, J)]   # dots psum chunks (bank-safe)


def build_nc(debug=False):
    nc = bacc.Bacc("TRN2", target_bir_lowering=False, debug=debug,
                   num_devices=NCORES)

    x8_e = nc.dram_tensor("x8", [P, KB * BL], F8, kind="ExternalInput")
    x8lo_e = nc.dram_tensor("x8lo", [P, KB * BL], F8, kind="ExternalInput")
    wsk8_e = nc.dram_tensor("wsk8", [P, KB * RSK], F8, kind="ExternalInput")
    d8_e = nc.dram_tensor("d8", [P, KB * J], F8, kind="ExternalInput")
    wv8_e = nc.dram_tensor("wv8", [CHB, P, KB * P], F8, kind="ExternalInput")
    wv8lo_e = nc.dram_tensor("wv8lo", [CHB, P, KB * P], F8, kind="ExternalInput")
    a8_e = nc.dram_tensor("a8", [P, 6 * CH], F8, kind="ExternalInput")
    evwb_e = nc.dram_tensor("evwb", [P, J], BF, kind="ExternalInput")
    bke_e = nc.dram_tensor("bke", [1, J], BF, kind="ExternalInput")
    wout_e = nc.dram_tensor("wout", [P, KB * NCLS], BF, kind="ExternalInput")
    bsk_e = nc.dram_tensor("bsk", [P, 1], F32, kind="ExternalInput")
    bv_e = nc.dram_tensor("bv", [P, CHB], F32, kind="ExternalInput")
    bout_e = nc.dram_tensor("bout", [NCLS, 1], F32, kind="ExternalInput")
    ident_e = nc.dram_tensor("ident", [P, P], BF, kind="ExternalInput")
    out_e = nc.dram_tensor("out", [NCLS, BL], F32, kind="ExternalOutput")

    with tile.TileContext(nc) as tc:
        from contextlib import ExitStack
        with ExitStack() as ctx:
            pers = ctx.enter_context(tc.tile_pool(name="pers", bufs=1))
            pmisc = ctx.enter_context(tc.tile_pool(name="pmisc", bufs=1, space="PSUM"))
            pkv = ctx.enter_context(tc.tile_pool(name="pkv", bufs=2, space="PSUM"))
            pdot = ctx.enter_context(tc.tile_pool(name="pdot", bufs=1, space="PSUM"))
            ptr = ctx.enter_context(tc.tile_pool(name="ptr", bufs=1, space="PSUM"))
            pfe = ctx.enter_context(tc.tile_pool(name="pfe", bufs=1, space="PSUM"))

            # ---- SBUF tiles ----
            x8_s = pers.tile([P, KB, BL], F8, tag="x8")
            x8lo_s = pers.tile([P, KB, BL], F8, tag="x8lo")
            wsk8_s = pers.tile([P, KB, RSK], F8, tag="wsk8")
            d8_s = pers.tile([P, KB, J], F8, tag="d8")
            wv8_s = pers.tile([P, CHB, KB, P], F8, tag="wv8")
            wv8lo_s = pers.tile([P, CHB, KB, P], F8, tag="wv8lo")
            a8_s = pers.tile([P, 6, CH], F8, tag="a8")
            evwb_s = pers.tile([P, J], BF, tag="evwb")
            bke_s = pers.tile([1, J], BF, tag="bke")
            wout_s = pers.tile([P, KB, NCLS], BF, tag="wout")
            bsk_s = pers.tile([P, 1], F32, tag="bsk")
            bv_s = pers.tile([P, CHB], F32, tag="bv")
            bout_s = pers.tile([NCLS, 1], F32, tag="bout")
            ident_s = pers.tile([P, P], BF, tag="ident")
            ones_s = pers.tile([P, 1], BF, tag="ones")
            ones1_s = pers.tile([1, P], BF, tag="ones1")
            scratch_s = pers.tile([1, 1], F32, tag="scratch")
            sk_s = pers.tile([P, BL], BF, tag="sk")
            sksq_s = pers.tile([P, BL], BF, tag="sksq")
            hv_s = pers.tile([P, CHB, BL], BF, tag="hv")
            hfe_s = pers.tile([P, CHB, BL], BF, tag="hfe")
            e_s = pers.tile([P, NB, J], BF, tag="e")
            tmp_s = pers.tile([P, J], BF, tag="tmp")
            u_s = pers.tile([P, NB, J], BF, tag="u")
            ut_s = pers.tile([P, 6, BL], F8, tag="ut")
            rinv_s = pers.tile([P, NB], F32, tag="rinv")
            rs1_s = pers.tile([P, NB], F32, tag="rs1")
            rs2_s = pers.tile([P, NB], F32, tag="rs2")
            magic_s = pers.tile([P, 1], mybir.dt.int32, tag="magic")
            s_s = pers.tile([P, NB * NCLS], BF, tag="s")
            num_s = pers.tile([P, NB * NCLS], BF, tag="num")
            sinv_s = pers.tile([P, NB * NCLS], F32, tag="sinv")
            t_s = pers.tile([P, NB * NCLS], F32, tag="t")
            g_s = pers.tile([P, NB * NCLS], F32, tag="g")
            gg_s = pers.tile([P, NB], F32, tag="gg")
            ginv_s = pers.tile([P, NB], F32, tag="ginv")
            c1_s = pers.tile([P, NB * NCLS], F32, tag="c1")
            c_s = pers.tile([P, NB * NCLS], F32, tag="c")
            out_sb = pers.tile([NCLS, BL], F32, tag="outsb")

            # ---- setup: memsets + pin the Exp ACT table before any evict ----
            nc.vector.memset(ones_s[:], 1.0)
            nc.vector.memset(ones1_s[:], 1.0)
            nc.vector.memset(magic_s[:], 0x5f3759df)
            nc.vector.memset(ut_s[:], 0.0)        # zero jb-5 pad partitions
            nc.vector.memset(scratch_s[:], 1.0)
            nc.scalar.activation(scratch_s[:], scratch_s[:], AF.Exp)

            # ---- DMA schedule (sync queue; DMA device is the critical
            # resource at ~22.6us busy). wv pairs last: each gates only one
            # v pass. ----
            nc.sync.dma_start(x8_s[:], x8_e.ap())
            nc.sync.dma_start(wsk8_s[:], wsk8_e.ap())
            nc.sync.dma_start(bke_s[:], bke_e.ap())
            nc.sync.dma_start(evwb_s[:], evwb_e.ap())
            nc.sync.dma_start(d8_s[:], d8_e.ap())
            nc.sync.dma_start(x8lo_s[:], x8lo_e.ap())
            nc.sync.dma_start(wout_s[:], wout_e.ap())
            for o in range(CHB):
                nc.sync.dma_start(wv8_s[:, o, :, :], wv8_e.ap()[o])
                nc.sync.dma_start(wv8lo_s[:, o, :, :], wv8lo_e.ap()[o])
                if o == 1:
                    nc.sync.dma_start(a8_s[:], a8_e.ap())
            nc.gpsimd.dma_start(bsk_s[:], bsk_e.ap())
            nc.gpsimd.dma_start(bv_s[:], bv_e.ap())
            nc.gpsimd.dma_start(bout_s[:], bout_e.ap())
            nc.gpsimd.dma_start(ident_s[:], ident_e.ap())

            # ---- phase 1: norm sketch: sk = 64*S*k, rinv = 1/||sk|| ----
            ps = pkv.tile([P, BL], F32, tag="pkv")
            for p in range(KP):
                nc.tensor.matmul(ps[0:RSK, :], wsk8_s[:, 2 * p:2 * p + 2, :],
                                 x8_s[:, 2 * p:2 * p + 2, :],
                                 start=(p == 0), stop=(p == KP - 1),
                                 perf_mode=DR)
            nc.scalar.activation(sk_s[0:RSK, :], ps[0:RSK, :], AF.Identity,
                                 bias=bsk_s[0:RSK, :])
            nc.vector.tensor_mul(sksq_s[0:RSK, :], sk_s[0:RSK, :],
                                 sk_s[0:RSK, :])
            ps2 = pmisc.tile([P, NB], F32, tag="misc")
            for bc in range(NB):
                nc.tensor.matmul(ps2[:, bc:bc + 1],
                                 sksq_s[0:RSK, bc * P:(bc + 1) * P],
                                 ones_s[0:RSK, :],
                                 start=True, stop=True)
                sq = rs1_s[:, bc:bc + 1]
                nc.vector.tensor_copy(sq, ps2[:, bc:bc + 1])
                y = rinv_s[:, bc:bc + 1]
                nc.vector.tensor_scalar(
                    y.bitcast(mybir.dt.int32), sq.bitcast(mybir.dt.int32),
                    1, None, op0=mybir.AluOpType.logical_shift_right)
                nc.vector.tensor_tensor(
                    out=y.bitcast(mybir.dt.int32), in0=magic_s[:],
                    in1=y.bitcast(mybir.dt.int32),
                    op=mybir.AluOpType.subtract)
                for _ in range(2):
                    t1 = rs2_s[:, bc:bc + 1]
                    nc.vector.tensor_mul(t1, y, y)
                    nc.vector.tensor_mul(t1, t1, sq)
                    nc.vector.tensor_scalar(t1, t1, -0.5, 1.5,
                                            op0=mybir.AluOpType.mult,
                                            op1=mybir.AluOpType.add)
                    nc.vector.tensor_mul(y, y, t1)

            # ---- phase 2: dots = x8 @ D8 (+bkE), chunk-major fp8 DR ----
            def dots(bc):
                psd = pdot.tile([P, J], F32, tag="pdot")
                for (c0, c1) in JC:
                    for p in range(KP):
                        nc.tensor.matmul(
                            psd[:, c0:c1],
                            x8_s[:, 2 * p:2 * p + 2, bc * P:bc * P + P],
                            d8_s[:, 2 * p:2 * p + 2, c0:c1],
                            start=(p == 0), stop=False, perf_mode=DR)
                    # += bkE (K=1 rank-1 broadcast matmul closes the group)
                    nc.tensor.matmul(psd[:, c0:c1], ones1_s[:],
                                     bke_s[:, c0:c1], start=False, stop=True)
                nc.scalar.activation(e_s[:, bc, 0:512], psd[:, 0:512], AF.Exp,
                                     scale=rinv_s[:, bc:bc + 1])
                nc.scalar.activation(e_s[:, bc, 512:J], psd[:, 512:J], AF.Exp,
                                     scale=rinv_s[:, bc:bc + 1])

            def softmax_chain(bc, eng):
                lp = nc.allow_low_precision(
                    reason="S/num feed softmax ratios; errors attenuate")
                lp.__enter__()
                # bc0 runs on DVE, bc1 on gpsimd: the two chains execute in
                # parallel so u1 lands ~2us earlier
                e_sl = e_s[:, bc, :]
                e3 = e_sl.rearrange("p (n m) -> p n m", m=M)
                ncls_sl = slice(bc * NCLS, (bc + 1) * NCLS)
                s2 = s_s[:, ncls_sl]
                eng.reduce_sum(s2, e3, axis=mybir.AxisListType.X)
                u_tmp = u_s[:, bc, :]
                eng.tensor_mul(u_tmp, e_sl, evwb_s[:])
                eng.reduce_sum(num_s[:, ncls_sl],
                               u_tmp.rearrange("p (n m) -> p n m", m=M),
                               axis=mybir.AxisListType.X)
                eng.reciprocal(sinv_s[:, ncls_sl], s2)
                eng.tensor_mul(t_s[:, ncls_sl], num_s[:, ncls_sl],
                               sinv_s[:, ncls_sl])
                nc.scalar.activation(g_s[:, ncls_sl], t_s[:, ncls_sl], AF.Exp)
                eng.reduce_sum(gg_s[:, bc:bc + 1], g_s[:, ncls_sl],
                               axis=mybir.AxisListType.X)
                eng.reciprocal(ginv_s[:, bc:bc + 1], gg_s[:, bc:bc + 1])
                # fold the u scale SU into ginv: c = g*sinv * (SU/G)
                eng.tensor_scalar(ginv_s[:, bc:bc + 1], ginv_s[:, bc:bc + 1],
                                  SU, None, op0=mybir.AluOpType.mult)
                eng.tensor_mul(c1_s[:, ncls_sl], g_s[:, ncls_sl],
                               sinv_s[:, ncls_sl])
                eng.tensor_scalar_mul(c_s[:, ncls_sl], c1_s[:, ncls_sl],
                                      ginv_s[:, bc:bc + 1])
                c_b = bass.AP(c_s.tensor, c_s[:, ncls_sl].offset,
                              c_s[:, ncls_sl].ap + [[0, M]])
                u3 = u_s[:, bc, :].rearrange("p (n m) -> p n m", m=M)
                eng.tensor_mul(u3, e3, c_b)
                lp.__exit__(None, None, None)

            # ---- out accumulator: block matmuls emitted as inputs land ----
            pso = pmisc.tile([NCLS, BL], F32, tag="misc")
            n_out_mm = [0]

            def out_mm(h_s, i):
                kb = i if h_s is hv_s else CHB + i
                nc.tensor.matmul(pso[:], wout_s[:, kb, :], h_s[:, i, :],
                                 start=(n_out_mm[0] == 0),
                                 stop=(n_out_mm[0] == KB - 1))
                n_out_mm[0] += 1

            def v_block(o):
                ps = pkv.tile([P, BL], F32, tag="pkv")
                n = 0
                for (wt, xt) in ((wv8_s, x8_s), (wv8_s, x8lo_s),
                                 (wv8lo_s, x8_s)):
                    for p in range(KP):
                        nc.tensor.matmul(ps[:], wt[:, o, 2 * p:2 * p + 2, :],
                                         xt[:, 2 * p:2 * p + 2, :],
                                         start=(n == 0), stop=(n == 3 * KP - 1),
                                         perf_mode=DR)
                        n += 1
                nc.scalar.activation(hv_s[:, o, :], ps[:], AF.Relu,
                                     scale=1.0 / SW, bias=bv_s[:, o:o + 1])

            def transpose_u(bc):
                def tgroup(grp):
                    pst = ptr.tile([P, 3 * P], BF, tag="ptr")
                    for t, jb in enumerate(grp):
                        w = P if jb < 5 else J - 5 * P
                        nc.tensor.transpose(
                            pst[:w, t * P:(t + 1) * P],
                            u_s[:, bc, jb * P:jb * P + w],
                            ident_s[:])
                    n = sum(1 for jb in grp if jb < 5)
                    base = ut_s[:, grp[0], bc * P:bc * P + P]
                    dst = bass.AP(ut_s.tensor, base.offset,
                                  [base.ap[0], [BL, n], base.ap[1]])
                    nc.vector.tensor_copy(
                        dst, pst[:, 0:n * P].rearrange("p (n q) -> p n q", q=P))
                    if n < len(grp):
                        jb = grp[n]
                        w = J - 5 * P
                        nc.scalar.activation(
                            ut_s[:w, jb, bc * P:bc * P + P],
                            pst[:w, n * P:(n + 1) * P], AF.Identity)
                tgroup((0, 1, 2))
                tgroup((3, 4, 5))

            def fe_all():
                for o in range(CHB):
                    acc = pfe.tile([P, BL], F32, tag=f"pfe{o % 2}")
                    for t in range(3):
                        nc.tensor.matmul(acc[:], a8_s[:, 2 * t:2 * t + 2,
                                                      o * P:(o + 1) * P],
                                         ut_s[:, 2 * t:2 * t + 2, :],
                                         start=(t == 0), stop=(t == 2),
                                         perf_mode=DR)
                    dst = hfe_s[:, o, :]
                    if o % 2 == 0:
                        nc.scalar.activation(dst, acc[:], AF.Relu,
                                             scale=1.0 / (SA * SU))
                    else:
                        nc.vector.tensor_scalar(dst, acc[:],
                                                1.0 / (SA * SU), 0.0,
                                                op0=mybir.AluOpType.mult,
                                                op1=mybir.AluOpType.max)

            # ---- main interleave: the whole dots/softmax/transpose/fE chain
            # runs before the wv stream thickens; v blocks then track DMA ----
            dots(0)
            softmax_chain(0, nc.vector)
            dots(1)
            softmax_chain(1, nc.vector)
            v_block(0)
            v_block(1)
            out_mm(hv_s, 0)
            transpose_u(0)
            transpose_u(1)
            fe_all()
            for i in range(CHB):
                out_mm(hfe_s, i)
            for o in range(2, CHB):
                v_block(o)
                out_mm(hv_s, o - 1)
            out_mm(hv_s, CHB - 1)

            # ---- +bout, DMA out ----
            nc.vector.tensor_scalar_add(out_sb[:], pso[:], bout_s[:, 0:1])
            nc.sync.dma_start(out_e.ap(), out_sb[:])

    nc.compile()
    return nc


def host_prep(x, static_feat, Wk, bk, Wv, bv, WEk, bEk, WEv, bEv, Ww, bw,
              Wout, bout):
    """Host-side fp32 precompute, fp8/bf16 quantization, per-core input maps."""
    EPS = 1e-8
    f32 = np.float32
    x = np.asarray(x, f32)
    static_feat = np.asarray(static_feat, f32)
    Wk, bk = np.asarray(Wk, f32), np.asarray(bk, f32)
    Wv, bv = np.asarray(Wv, f32), np.asarray(bv, f32)
    Wout, bout = np.asarray(Wout, f32), np.asarray(bout, f32)

    Ek = np.einsum('oc,ncm->nom', np.asarray(WEk, f32), static_feat,
                   optimize=True) + np.asarray(bEk, f32)[None, :, None]
    Ev = np.einsum('oc,ncm->nom', np.asarray(WEv, f32), static_feat,
                   optimize=True) + np.asarray(bEv, f32)[None, :, None]
    Ekn = Ek / np.maximum(np.linalg.norm(Ek, axis=1, keepdims=True), EPS)
    Ekn_mat = Ekn.transpose(1, 0, 2).reshape(CH, J)          # [CH, 672]
    A_mat = Ev.transpose(0, 2, 1).reshape(J, CH)             # [672, CH]
    evwb = np.einsum('nom,o->nm', Ev, np.asarray(Ww, f32)[0]).reshape(J)

    # norm sketch + folded dots
    S = np.random.RandomState(0).randn(RSK, CH).astype(f32) / np.sqrt(RSK)
    W_sk = S @ Wk                                            # [128, CIN]
    b_sk = S @ bk
    D = Wk.T @ Ekn_mat                                       # [CIN, J]
    bkE = bk @ Ekn_mat                                       # [J]

    def cinlayout(w, width):    # [CIN, width] -> [P, KB*width]
        return np.ascontiguousarray(
            w.reshape(KB, P, width).transpose(1, 0, 2).reshape(P, KB * width))

    wsk8_h = cinlayout((W_sk.T * SD).astype(f8), RSK)
    d8_h = cinlayout((D * SD).astype(f8), J)

    def wlayout(w):     # [CIN, OCH] f8 -> dram [OCH/P, P, KB*P]
        och = w.shape[1]
        return np.ascontiguousarray(
            w.reshape(KB, P, och // P, P).transpose(2, 1, 0, 3)
            .reshape(och // P, P, KB * P))

    wv_s = Wv.T * SW
    wv8_f = wv_s.astype(f8)
    wv8lo_f = (wv_s - wv8_f.astype(f32)).astype(f8)
    wv8_h = wlayout(wv8_f)
    wv8lo_h = wlayout(wv8lo_f)

    a_pad = np.zeros((6 * P, CH), f32)
    a_pad[:J] = A_mat * SA
    a8_h = np.ascontiguousarray(
        a_pad.astype(f8).reshape(6, P, CH).transpose(1, 0, 2).reshape(P, 6 * CH))
    evwb_h = np.ascontiguousarray(
        np.broadcast_to(evwb.astype(bf16)[None, :], (P, J)))
    bke_h = (bkE * SD).astype(bf16).reshape(1, J)
    wout_h = np.ascontiguousarray(
        Wout.T.reshape(KB, P, NCLS).transpose(1, 0, 2).reshape(
            P, KB * NCLS).astype(bf16))
    bsk_h = np.zeros((P, 1), f32)
    bsk_h[:RSK, 0] = b_sk * SD
    bv_h = np.ascontiguousarray(bv.reshape(CHB, P).T)
    bout_h = bout.reshape(NCLS, 1)
    ident_h = np.eye(P, dtype=bf16)

    xT = np.ascontiguousarray(x[:, -1, :].T)                 # [CIN, B]
    x8_f = xT.astype(f8)
    x8lo_f = (xT - x8_f.astype(f32)).astype(f8)

    shared = dict(wsk8=wsk8_h, d8=d8_h, wv8=wv8_h, wv8lo=wv8lo_h, a8=a8_h,
                  evwb=evwb_h, bke=bke_h, wout=wout_h, bsk=bsk_h, bv=bv_h,
                  bout=bout_h, ident=ident_h)
    in_maps = []
    for c in range(NCORES):
        sl = slice(c * BL, (c + 1) * BL)

        def xlayout(xf):
            return np.ascontiguousarray(
                xf[:, sl].reshape(KB, P, BL).transpose(1, 0, 2)
                .reshape(P, KB * BL))
        in_maps.append(dict(x8=xlayout(x8_f), x8lo=xlayout(x8lo_f), **shared))
    return in_maps


_NC_CACHE = {}


def get_nc(debug=False):
    if debug not in _NC_CACHE:
        _NC_CACHE[debug] = build_nc(debug=debug)
    return _NC_CACHE[debug]


def kernel(**inputs) -> np.ndarray:
    nc = get_nc()
    in_maps = host_prep(**inputs)
    res = run_bass_kernel_spmd(nc, in_maps, list(range(NCORES)))
    out = np.empty((B, NCLS, 1), dtype=np.float32)
    for c in range(NCORES):
        out[c * BL:(c + 1) * BL, :, 0] = res.results[c]["out"].T
    return out


# revision 38
# speedup vs baseline: 1.0149x; 1.0094x over previous
"""Trainium2 Bass kernel for nn_Colar_static (retrieval_knn).

Sharding: data-parallel over batch B=2048 across 8 NeuronCores (256 rows each).
Weights/exemplars replicated, precomputed + quantized on host.

Design (vs the bf16 baseline at 53.3us):
  * Every large matmul is fp8e4m3 with the DoubleRow perf mode (K=256 per
    instruction, 0.5 cycles/row) -> 4x bf16 MAC rate and 1-byte weights
    (the kernel is DMA-bound: all DMA serializes at ~332 GB/s).
  * dots = x @ (Wk^T Ekn) directly: Wk is folded into the exemplars on the
    host, so the 2MB Wk and 0.7MB Ekn never ship; only D8 [CIN,672] (1.4MB).
  * ||k|| (softmax temperature only) via a random sketch: ||S k|| with
    S [128,1024] Gaussian, W_sk = S Wk [128, CIN] fp8 (0.25MB). The 5% norm
    error is invisible downstream (validated: rel err 3.21e-3, same as the
    exact-norm pipeline, because cos logits are tiny and softmax-smoothed).
  * v  = x8@Wv8hi + x8lo@Wv8hi + x8@Wv8lo   3-pass residual-compensated fp8
    (v dominates the output; plain fp8 fails at 3.3e-2).
  * fE = A8^T @ ut8 (fp8 DR);  out = Wout^T @ [hv;hfe] in bf16 (tiny).

Scales (all folded, no extra device work): D,W_sk x64; Wv x32; A x16; u x256.
The sketch scale cancels: rinv = rsqrt(sum((64 S k)^2)) = 1/(64||Sk||) and
dots are x64, so exp(dots*rinv) = exp(cos).

Rel err vs fp32 reference ~3.2e-3 (numpy-sim validated; gate is 2e-2).
"""

import numpy as np
import ml_dtypes

import concourse.bass as bass
import concourse.bacc as bacc
import concourse.mybir as mybir
import concourse.tile as tile
from concourse.bass_utils import run_bass_kernel_spmd

AF = mybir.ActivationFunctionType
BF = mybir.dt.bfloat16
F8 = mybir.dt.float8e4
F32 = mybir.dt.float32
DR = mybir.MatmulPerfMode.DoubleRow
bf16 = ml_dtypes.bfloat16
f8 = ml_dtypes.float8_e4m3

# Problem constants (hardcoded; kernel.py must be self-contained)
B, T, CIN, CH, M, NCLS = 2048, 8, 2048, 1024, 32, 21
NCORES = 8
BL = B // NCORES          # 256 batch rows per core
J = NCLS * M              # 672
P = 128
KB = CIN // P             # 16 contraction blocks over CIN
KP = KB // 2              # 8 DoubleRow pairs over CIN
CHB = CH // P             # 8 blocks over CH
NB = BL // P              # 2 batch chunks of 128
RSK = 64                  # norm-sketch rank
SD, SW, SA, SU = 64.0, 32.0, 16.0, 256.0
JC = [(0, 256), (256, 512), (512, J)]   # dots psum chunks (bank-safe)


def build_nc(debug=False):
    nc = bacc.Bacc("TRN2", target_bir_lowering=False, debug=debug,
                   num_devices=NCORES)

    x8_e = nc.dram_tensor("x8", [P, KB * BL], F8, kind="ExternalInput")
    x8lo_e = nc.dram_tensor("x8lo", [P, KB * BL], F8, kind="ExternalInput")
    wsk8_e = nc.dram_tensor("wsk8", [P, KB * RSK], F8, kind="ExternalInput")
    d8_e = nc.dram_tensor("d8", [P, KB * J], F8, kind="ExternalInput")
    wv8_e = nc.dram_tensor("wv8", [CHB, P, KB * P], F8, kind="ExternalInput")
    wv8lo_e = nc.dram_tensor("wv8lo", [CHB, P, KB * P], F8, kind="ExternalInput")
    a8_e = nc.dram_tensor("a8", [P, 6 * CH], F8, kind="ExternalInput")
    evwb_e = nc.dram_tensor("evwb", [P, J], BF, kind="ExternalInput")
    bke_e = nc.dram_tensor("bke", [1, J], BF, kind="ExternalInput")
    wout_e = nc.dram_tensor("wout", [P, KB * NCLS], BF, kind="ExternalInput")
    bsk_e = nc.dram_tensor("bsk", [P, 1], F32, kind="ExternalInput")
    bv_e = nc.dram_tensor("bv", [P, CHB], F32, kind="ExternalInput")
    bout_e = nc.dram_tensor("bout", [NCLS, 1], F32, kind="ExternalInput")
    ident_e = nc.dram_tensor("ident", [P, P], BF, kind="ExternalInput")
    out_e = nc.dram_tensor("out", [NCLS, BL], F32, kind="ExternalOutput")

    with tile.TileContext(nc) as tc:
        from contextlib import ExitStack
        with ExitStack() as ctx:
            pers = ctx.enter_context(tc.tile_pool(name="pers", bufs=1))
            pmisc = ctx.enter_context(tc.tile_pool(name="pmisc", bufs=1, space="PSUM"))
            pkv = ctx.enter_context(tc.tile_pool(name="pkv", bufs=2, space="PSUM"))
            pdot = ctx.enter_context(tc.tile_pool(name="pdot", bufs=1, space="PSUM"))
            ptr = ctx.enter_context(tc.tile_pool(name="ptr", bufs=1, space="PSUM"))
            pfe = ctx.enter_context(tc.tile_pool(name="pfe", bufs=1, space="PSUM"))

            # ---- SBUF tiles ----
            x8_s = pers.tile([P, KB, BL], F8, tag="x8")
            x8lo_s = pers.tile([P, KB, BL], F8, tag="x8lo")
            wsk8_s = pers.tile([P, KB, RSK], F8, tag="wsk8")
            d8_s = pers.tile([P, KB, J], F8, tag="d8")
            wv8_s = pers.tile([P, CHB, KB, P], F8, tag="wv8")
            wv8lo_s = pers.tile([P, CHB, KB, P], F8, tag="wv8lo")
            a8_s = pers.tile([P, 6, CH], F8, tag="a8")
            evwb_s = pers.tile([P, J], BF, tag="evwb")
            bke_s = pers.tile([1, J], BF, tag="bke")
            wout_s = pers.tile([P, KB, NCLS], BF, tag="wout")
            bsk_s = pers.tile([P, 1], F32, tag="bsk")
            bv_s = pers.tile([P, CHB], F32, tag="bv")
            bout_s = pers.tile([NCLS, 1], F32, tag="bout")
            ident_s = pers.tile([P, P], BF, tag="ident")
            ones_s = pers.tile([P, 1], BF, tag="ones")
            ones1_s = pers.tile([1, P], BF, tag="ones1")
            scratch_s = pers.tile([1, 1], F32, tag="scratch")
            sk_s = pers.tile([P, BL], BF, tag="sk")
            sksq_s = pers.tile([P, BL], BF, tag="sksq")
            hv_s = pers.tile([P, CHB, BL], BF, tag="hv")
            hfe_s = pers.tile([P, CHB, BL], BF, tag="hfe")
            e_s = pers.tile([P, NB, J], BF, tag="e")
            tmp_s = pers.tile([P, J], BF, tag="tmp")
            u_s = pers.tile([P, NB, J], BF, tag="u")
            ut_s = pers.tile([P, 6, BL], F8, tag="ut")
            rinv_s = pers.tile([P, NB], F32, tag="rinv")
            rs1_s = pers.tile([P, NB], F32, tag="rs1")
            rs2_s = pers.tile([P, NB], F32, tag="rs2")
            magic_s = pers.tile([P, 1], mybir.dt.int32, tag="magic")
            s_s = pers.tile([P, NB * NCLS], BF, tag="s")
            num_s = pers.tile([P, NB * NCLS], BF, tag="num")
            sinv_s = pers.tile([P, NB * NCLS], F32, tag="sinv")
            t_s = pers.tile([P, NB * NCLS], F32, tag="t")
            g_s = pers.tile([P, NB * NCLS], F32, tag="g")
            gg_s = pers.tile([P, NB], F32, tag="gg")
            ginv_s = pers.tile([P, NB], F32, tag="ginv")
            c1_s = pers.tile([P, NB * NCLS], F32, tag="c1")
            c_s = pers.tile([P, NB * NCLS], F32, tag="c")
            out_sb = pers.tile([NCLS, BL], F32, tag="outsb")

            # ---- setup: memsets + pin the Exp ACT table before any evict ----
            nc.vector.memset(ones_s[:], 1.0)
            nc.vector.memset(ones1_s[:], 1.0)
            nc.vector.memset(magic_s[:], 0x5f3759df)
            nc.vector.memset(ut_s[:], 0.0)        # zero jb-5 pad partitions
            nc.vector.memset(scratch_s[:], 1.0)
            nc.scalar.activation(scratch_s[:], scratch_s[:], AF.Exp)

            # ---- DMA schedule (sync queue; DMA device is the critical
            # resource at ~22.6us busy). wv pairs last: each gates only one
            # v pass. ----
            nc.sync.dma_start(x8_s[:], x8_e.ap())
            nc.sync.dma_start(wsk8_s[:], wsk8_e.ap())
            nc.sync.dma_start(bke_s[:], bke_e.ap())
            nc.sync.dma_start(evwb_s[:], evwb_e.ap())
            nc.sync.dma_start(d8_s[:], d8_e.ap())
            nc.sync.dma_start(x8lo_s[:], x8lo_e.ap())
            nc.sync.dma_start(wout_s[:], wout_e.ap())
            for o in range(CHB):
                nc.sync.dma_start(wv8_s[:, o, :, :], wv8_e.ap()[o])
                nc.sync.dma_start(wv8lo_s[:, o, :, :], wv8lo_e.ap()[o])
                if o == 1:
                    nc.sync.dma_start(a8_s[:], a8_e.ap())
            nc.gpsimd.dma_start(bsk_s[:], bsk_e.ap())
            nc.gpsimd.dma_start(bv_s[:], bv_e.ap())
            nc.gpsimd.dma_start(bout_s[:], bout_e.ap())
            nc.gpsimd.dma_start(ident_s[:], ident_e.ap())

            # ---- phase 1: norm sketch: sk = 64*S*k, rinv = 1/||sk|| ----
            ps = pkv.tile([P, BL], F32, tag="pkv")
            for p in range(KP):
                nc.tensor.matmul(ps[0:RSK, :], wsk8_s[:, 2 * p:2 * p + 2, :],
                                 x8_s[:, 2 * p:2 * p + 2, :],
                                 start=(p == 0), stop=(p == KP - 1),
                                 perf_mode=DR)
            nc.scalar.activation(sk_s[0:RSK, :], ps[0:RSK, :], AF.Identity,
                                 bias=bsk_s[0:RSK, :])
            nc.vector.tensor_mul(sksq_s[0:RSK, :], sk_s[0:RSK, :],
                                 sk_s[0:RSK, :])
            ps2 = pmisc.tile([P, NB], F32, tag="misc")
            for bc in range(NB):
                nc.tensor.matmul(ps2[:, bc:bc + 1],
                                 sksq_s[0:RSK, bc * P:(bc + 1) * P],
                                 ones_s[0:RSK, :],
                                 start=True, stop=True)
                sq = rs1_s[:, bc:bc + 1]
                nc.vector.tensor_copy(sq, ps2[:, bc:bc + 1])
                y = rinv_s[:, bc:bc + 1]
                nc.vector.tensor_scalar(
                    y.bitcast(mybir.dt.int32), sq.bitcast(mybir.dt.int32),
                    1, None, op0=mybir.AluOpType.logical_shift_right)
                nc.vector.tensor_tensor(
                    out=y.bitcast(mybir.dt.int32), in0=magic_s[:],
                    in1=y.bitcast(mybir.dt.int32),
                    op=mybir.AluOpType.subtract)
                for _ in range(2):
                    t1 = rs2_s[:, bc:bc + 1]
                    nc.vector.tensor_mul(t1, y, y)
                    nc.vector.tensor_mul(t1, t1, sq)
                    nc.vector.tensor_scalar(t1, t1, -0.5, 1.5,
                                            op0=mybir.AluOpType.mult,
                                            op1=mybir.AluOpType.add)
                    nc.vector.tensor_mul(y, y, t1)

            # ---- phase 2: dots = x8 @ D8 (+bkE), chunk-major fp8 DR ----
            def dots(bc):
                psd = pdot.tile([P, J], F32, tag="pdot")
                for (c0, c1) in JC:
                    for p in range(KP):
                        nc.tensor.matmul(
                            psd[:, c0:c1],
                            x8_s[:, 2 * p:2 * p + 2, bc * P:bc * P + P],
                            d8_s[:, 2 * p:2 * p + 2, c0:c1],
                            start=(p == 0), stop=False, perf_mode=DR)
                    # += bkE (K=1 rank-1 broadcast matmul closes the group)
                    nc.tensor.matmul(psd[:, c0:c1], ones1_s[:],
                                     bke_s[:, c0:c1], start=False, stop=True)
                nc.scalar.activation(e_s[:, bc, 0:512], psd[:, 0:512], AF.Exp,
                                     scale=rinv_s[:, bc:bc + 1])
                nc.scalar.activation(e_s[:, bc, 512:J], psd[:, 512:J], AF.Exp,
                                     scale=rinv_s[:, bc:bc + 1])

            def softmax_chain(bc, eng):
                lp = nc.allow_low_precision(
                    reason="S/num feed softmax ratios; errors attenuate")
                lp.__enter__()
                # bc0 runs on DVE, bc1 on gpsimd: the two chains execute in
                # parallel so u1 lands ~2us earlier
                e_sl = e_s[:, bc, :]
                e3 = e_sl.rearrange("p (n m) -> p n m", m=M)
                ncls_sl = slice(bc * NCLS, (bc + 1) * NCLS)
                s2 = s_s[:, ncls_sl]
                eng.reduce_sum(s2, e3, axis=mybir.AxisListType.X)
                u_tmp = u_s[:, bc, :]
                eng.tensor_mul(u_tmp, e_sl, evwb_s[:])
                eng.reduce_sum(num_s[:, ncls_sl],
                               u_tmp.rearrange("p (n m) -> p n m", m=M),
                               axis=mybir.AxisListType.X)
                eng.reciprocal(sinv_s[:, ncls_sl], s2)
                eng.tensor_mul(t_s[:, ncls_sl], num_s[:, ncls_sl],
                               sinv_s[:, ncls_sl])
                nc.scalar.activation(g_s[:, ncls_sl], t_s[:, ncls_sl], AF.Exp)
                eng.reduce_sum(gg_s[:, bc:bc + 1], g_s[:, ncls_sl],
                               axis=mybir.AxisListType.X)
                eng.reciprocal(ginv_s[:, bc:bc + 1], gg_s[:, bc:bc + 1])
                # fold the u scale SU into ginv: c = g*sinv * (SU/G)
                eng.tensor_scalar(ginv_s[:, bc:bc + 1], ginv_s[:, bc:bc + 1],
                                  SU, None, op0=mybir.AluOpType.mult)
                eng.tensor_mul(c1_s[:, ncls_sl], g_s[:, ncls_sl],
                               sinv_s[:, ncls_sl])
                eng.tensor_scalar_mul(c_s[:, ncls_sl], c1_s[:, ncls_sl],
                                      ginv_s[:, bc:bc + 1])
                c_b = bass.AP(c_s.tensor, c_s[:, ncls_sl].offset,
                              c_s[:, ncls_sl].ap + [[0, M]])
                u3 = u_s[:, bc, :].rearrange("p (n m) -> p n m", m=M)
                eng.tensor_mul(u3, e3, c_b)
                lp.__exit__(None, None, None)

            # ---- out accumulator: block matmuls emitted as inputs land ----
            pso = pmisc.tile([NCLS, BL], F32, tag="misc")
            n_out_mm = [0]

            def out_mm(h_s, i):
                kb = i if h_s is hv_s else CHB + i
                nc.tensor.matmul(pso[:], wout_s[:, kb, :], h_s[:, i, :],
                                 start=(n_out_mm[0] == 0),
                                 stop=(n_out_mm[0] == KB - 1))
                n_out_mm[0] += 1

            def v_block(o):
                ps = pkv.tile([P, BL], F32, tag="pkv")
                n = 0
                for (wt, xt) in ((wv8_s, x8_s), (wv8_s, x8lo_s),
                                 (wv8lo_s, x8_s)):
                    for p in range(KP):
                        nc.tensor.matmul(ps[:], wt[:, o, 2 * p:2 * p + 2, :],
                                         xt[:, 2 * p:2 * p + 2, :],
                                         start=(n == 0), stop=(n == 3 * KP - 1),
                                         perf_mode=DR)
                        n += 1
                nc.scalar.activation(hv_s[:, o, :], ps[:], AF.Relu,
                                     scale=1.0 / SW, bias=bv_s[:, o:o + 1])

            def transpose_u(bc):
                def tgroup(grp, pool, ptag):
                    pst = pool.tile([P, 3 * P], BF, tag=ptag)
                    for t, jb in enumerate(grp):
                        w = P if jb < 5 else J - 5 * P
                        nc.tensor.transpose(
                            pst[:w, t * P:(t + 1) * P],
                            u_s[:, bc, jb * P:jb * P + w],
                            ident_s[:])
                    n = sum(1 for jb in grp if jb < 5)
                    base = ut_s[:, grp[0], bc * P:bc * P + P]
                    dst = bass.AP(ut_s.tensor, base.offset,
                                  [base.ap[0], [BL, n], base.ap[1]])
                    nc.vector.tensor_copy(
                        dst, pst[:, 0:n * P].rearrange("p (n q) -> p n q", q=P))
                    if n < len(grp):
                        jb = grp[n]
                        w = J - 5 * P
                        nc.scalar.activation(
                            ut_s[:w, jb, bc * P:bc * P + P],
                            pst[:w, n * P:(n + 1) * P], AF.Identity)
                # alternate psum pools so the four transpose groups pipeline
                tgroup((0, 1, 2), ptr, "ptr")
                tgroup((3, 4, 5), pfe, f"pfe{bc}")

            def fe_all():
                for o in range(CHB):
                    acc = pfe.tile([P, BL], F32, tag=f"pfe{o % 2}")
                    for t in range(3):
                        nc.tensor.matmul(acc[:], a8_s[:, 2 * t:2 * t + 2,
                                                      o * P:(o + 1) * P],
                                         ut_s[:, 2 * t:2 * t + 2, :],
                                         start=(t == 0), stop=(t == 2),
                                         perf_mode=DR)
                    dst = hfe_s[:, o, :]
                    if o % 2 == 0:
                        nc.scalar.activation(dst, acc[:], AF.Relu,
                                             scale=1.0 / (SA * SU))
                    else:
                        nc.vector.tensor_scalar(dst, acc[:],
                                                1.0 / (SA * SU), 0.0,
                                                op0=mybir.AluOpType.mult,
                                                op1=mybir.AluOpType.max)

            # ---- main interleave: the whole dots/softmax/transpose/fE chain
            # runs before the wv stream thickens; v blocks then track DMA ----
            dots(0)
            softmax_chain(0, nc.vector)
            dots(1)
            softmax_chain(1, nc.vector)
            v_block(0)
            v_block(1)
            out_mm(hv_s, 0)
            transpose_u(0)
            transpose_u(1)
            fe_all()
            for i in range(CHB):
                out_mm(hfe_s, i)
            for o in range(2, CHB):
                v_block(o)
                out_mm(hv_s, o - 1)
            out_mm(hv_s, CHB - 1)

            # ---- +bout, DMA out ----
            nc.vector.tensor_scalar_add(out_sb[:], pso[:], bout_s[:, 0:1])
            nc.sync.dma_start(out_e.ap(), out_sb[:])

    nc.compile()
    return nc


def host_prep(x, static_feat, Wk, bk, Wv, bv, WEk, bEk, WEv, bEv, Ww, bw,
              Wout, bout):
    """Host-side fp32 precompute, fp8/bf16 quantization, per-core input maps."""
    EPS = 1e-8
    f32 = np.float32
    x = np.asarray(x, f32)
    static_feat = np.asarray(static_feat, f32)
    Wk, bk = np.asarray(Wk, f32), np.asarray(bk, f32)
    Wv, bv = np.asarray(Wv, f32), np.asarray(bv, f32)
    Wout, bout = np.asarray(Wout, f32), np.asarray(bout, f32)

    Ek = np.einsum('oc,ncm->nom', np.asarray(WEk, f32), static_feat,
                   optimize=True) + np.asarray(bEk, f32)[None, :, None]
    Ev = np.einsum('oc,ncm->nom', np.asarray(WEv, f32), static_feat,
                   optimize=True) + np.asarray(bEv, f32)[None, :, None]
    Ekn = Ek / np.maximum(np.linalg.norm(Ek, axis=1, keepdims=True), EPS)
    Ekn_mat = Ekn.transpose(1, 0, 2).reshape(CH, J)          # [CH, 672]
    A_mat = Ev.transpose(0, 2, 1).reshape(J, CH)             # [672, CH]
    evwb = np.einsum('nom,o->nm', Ev, np.asarray(Ww, f32)[0]).reshape(J)

    # norm sketch + folded dots
    S = np.random.RandomState(0).randn(RSK, CH).astype(f32) / np.sqrt(RSK)
    W_sk = S @ Wk                                            # [128, CIN]
    b_sk = S @ bk
    D = Wk.T @ Ekn_mat                                       # [CIN, J]
    bkE = bk @ Ekn_mat                                       # [J]

    def cinlayout(w, width):    # [CIN, width] -> [P, KB*width]
        return np.ascontiguousarray(
            w.reshape(KB, P, width).transpose(1, 0, 2).reshape(P, KB * width))

    wsk8_h = cinlayout((W_sk.T * SD).astype(f8), RSK)
    d8_h = cinlayout((D * SD).astype(f8), J)

    def wlayout(w):     # [CIN, OCH] f8 -> dram [OCH/P, P, KB*P]
        och = w.shape[1]
        return np.ascontiguousarray(
            w.reshape(KB, P, och // P, P).transpose(2, 1, 0, 3)
            .reshape(och // P, P, KB * P))

    wv_s = Wv.T * SW
    wv8_f = wv_s.astype(f8)
    wv8lo_f = (wv_s - wv8_f.astype(f32)).astype(f8)
    wv8_h = wlayout(wv8_f)
    wv8lo_h = wlayout(wv8lo_f)

    a_pad = np.zeros((6 * P, CH), f32)
    a_pad[:J] = A_mat * SA
    a8_h = np.ascontiguousarray(
        a_pad.astype(f8).reshape(6, P, CH).transpose(1, 0, 2).reshape(P, 6 * CH))
    evwb_h = np.ascontiguousarray(
        np.broadcast_to(evwb.astype(bf16)[None, :], (P, J)))
    bke_h = (bkE * SD).astype(bf16).reshape(1, J)
    wout_h = np.ascontiguousarray(
        Wout.T.reshape(KB, P, NCLS).transpose(1, 0, 2).reshape(
            P, KB * NCLS).astype(bf16))
    bsk_h = np.zeros((P, 1), f32)
    bsk_h[:RSK, 0] = b_sk * SD
    bv_h = np.ascontiguousarray(bv.reshape(CHB, P).T)
    bout_h = bout.reshape(NCLS, 1)
    ident_h = np.eye(P, dtype=bf16)

    xT = np.ascontiguousarray(x[:, -1, :].T)                 # [CIN, B]
    x8_f = xT.astype(f8)
    x8lo_f = (xT - x8_f.astype(f32)).astype(f8)

    shared = dict(wsk8=wsk8_h, d8=d8_h, wv8=wv8_h, wv8lo=wv8lo_h, a8=a8_h,
                  evwb=evwb_h, bke=bke_h, wout=wout_h, bsk=bsk_h, bv=bv_h,
                  bout=bout_h, ident=ident_h)
    in_maps = []
    for c in range(NCORES):
        sl = slice(c * BL, (c + 1) * BL)

        def xlayout(xf):
            return np.ascontiguousarray(
                xf[:, sl].reshape(KB, P, BL).transpose(1, 0, 2)
                .reshape(P, KB * BL))
        in_maps.append(dict(x8=xlayout(x8_f), x8lo=xlayout(x8lo_f), **shared))
    return in_maps


_NC_CACHE = {}


def get_nc(debug=False):
    if debug not in _NC_CACHE:
        _NC_CACHE[debug] = build_nc(debug=debug)
    return _NC_CACHE[debug]


def kernel(**inputs) -> np.ndarray:
    nc = get_nc()
    in_maps = host_prep(**inputs)
    res = run_bass_kernel_spmd(nc, in_maps, list(range(NCORES)))
    out = np.empty((B, NCLS, 1), dtype=np.float32)
    for c in range(NCORES):
        out[c * BL:(c + 1) * BL, :, 0] = res.results[c]["out"].T
    return out


# revision 39
# speedup vs baseline: 1.0176x; 1.0026x over previous
"""Trainium2 Bass kernel for nn_Colar_static (retrieval_knn).

Sharding: data-parallel over batch B=2048 across 8 NeuronCores (256 rows each).
Weights/exemplars replicated, precomputed + quantized on host.

Design (vs the bf16 baseline at 53.3us):
  * Every large matmul is fp8e4m3 with the DoubleRow perf mode (K=256 per
    instruction, 0.5 cycles/row) -> 4x bf16 MAC rate and 1-byte weights
    (the kernel is DMA-bound: all DMA serializes at ~332 GB/s).
  * dots = x @ (Wk^T Ekn) directly: Wk is folded into the exemplars on the
    host, so the 2MB Wk and 0.7MB Ekn never ship; only D8 [CIN,672] (1.4MB).
  * ||k|| (softmax temperature only) via a random sketch: ||S k|| with
    S [128,1024] Gaussian, W_sk = S Wk [128, CIN] fp8 (0.25MB). The 5% norm
    error is invisible downstream (validated: rel err 3.21e-3, same as the
    exact-norm pipeline, because cos logits are tiny and softmax-smoothed).
  * v  = x8@Wv8hi + x8lo@Wv8hi + x8@Wv8lo   3-pass residual-compensated fp8
    (v dominates the output; plain fp8 fails at 3.3e-2).
  * fE = A8^T @ ut8 (fp8 DR);  out = Wout^T @ [hv;hfe] in bf16 (tiny).

Scales (all folded, no extra device work): D,W_sk x64; Wv x32; A x16; u x256.
The sketch scale cancels: rinv = rsqrt(sum((64 S k)^2)) = 1/(64||Sk||) and
dots are x64, so exp(dots*rinv) = exp(cos).

Rel err vs fp32 reference ~3.2e-3 (numpy-sim validated; gate is 2e-2).
"""

import numpy as np
import ml_dtypes

import concourse.bass as bass
import concourse.bacc as bacc
import concourse.mybir as mybir
import concourse.tile as tile
from concourse.bass_utils import run_bass_kernel_spmd

AF = mybir.ActivationFunctionType
BF = mybir.dt.bfloat16
F8 = mybir.dt.float8e4
F32 = mybir.dt.float32
DR = mybir.MatmulPerfMode.DoubleRow
bf16 = ml_dtypes.bfloat16
f8 = ml_dtypes.float8_e4m3

# Problem constants (hardcoded; kernel.py must be self-contained)
B, T, CIN, CH, M, NCLS = 2048, 8, 2048, 1024, 32, 21
NCORES = 8
BL = B // NCORES          # 256 batch rows per core
J = NCLS * M              # 672
P = 128
KB = CIN // P             # 16 contraction blocks over CIN
KP = KB // 2              # 8 DoubleRow pairs over CIN
CHB = CH // P             # 8 blocks over CH
NB = BL // P              # 2 batch chunks of 128
RSK = 64                  # norm-sketch rank
SD, SW, SA, SU = 64.0, 32.0, 16.0, 256.0
JC = [(0, 256), (256, 512), (512, J)]   # dots psum chunks (bank-safe)


def build_nc(debug=False):
    nc = bacc.Bacc("TRN2", target_bir_lowering=False, debug=debug,
                   num_devices=NCORES)

    x8_e = nc.dram_tensor("x8", [P, KB * BL], F8, kind="ExternalInput")
    x8lo_e = nc.dram_tensor("x8lo", [P, KB * BL], F8, kind="ExternalInput")
    wsk8_e = nc.dram_tensor("wsk8", [P, KB * RSK], F8, kind="ExternalInput")
    d8_e = nc.dram_tensor("d8", [P, KB * J], F8, kind="ExternalInput")
    wv8_e = nc.dram_tensor("wv8", [CHB, P, KB * P], F8, kind="ExternalInput")
    wv8lo_e = nc.dram_tensor("wv8lo", [CHB, P, KB * P], F8, kind="ExternalInput")
    a8_e = nc.dram_tensor("a8", [P, 6 * CH], F8, kind="ExternalInput")
    evwb_e = nc.dram_tensor("evwb", [P, J], BF, kind="ExternalInput")
    bke_e = nc.dram_tensor("bke", [1, J], BF, kind="ExternalInput")
    wout_e = nc.dram_tensor("wout", [P, KB * NCLS], BF, kind="ExternalInput")
    bsk_e = nc.dram_tensor("bsk", [P, 1], F32, kind="ExternalInput")
    bv_e = nc.dram_tensor("bv", [P, CHB], F32, kind="ExternalInput")
    bout_e = nc.dram_tensor("bout", [NCLS, 1], F32, kind="ExternalInput")
    ident_e = nc.dram_tensor("ident", [P, P], BF, kind="ExternalInput")
    out_e = nc.dram_tensor("out", [NCLS, BL], F32, kind="ExternalOutput")

    with tile.TileContext(nc) as tc:
        from contextlib import ExitStack
        with ExitStack() as ctx:
            pers = ctx.enter_context(tc.tile_pool(name="pers", bufs=1))
            pmisc = ctx.enter_context(tc.tile_pool(name="pmisc", bufs=1, space="PSUM"))
            pkv = ctx.enter_context(tc.tile_pool(name="pkv", bufs=2, space="PSUM"))
            pdot = ctx.enter_context(tc.tile_pool(name="pdot", bufs=1, space="PSUM"))
            ptr = ctx.enter_context(tc.tile_pool(name="ptr", bufs=1, space="PSUM"))
            pfe = ctx.enter_context(tc.tile_pool(name="pfe", bufs=1, space="PSUM"))

            # ---- SBUF tiles ----
            x8_s = pers.tile([P, KB, BL], F8, tag="x8")
            x8lo_s = pers.tile([P, KB, BL], F8, tag="x8lo")
            wsk8_s = pers.tile([P, KB, RSK], F8, tag="wsk8")
            d8_s = pers.tile([P, KB, J], F8, tag="d8")
            wv8_s = pers.tile([P, CHB, KB, P], F8, tag="wv8")
            wv8lo_s = pers.tile([P, CHB, KB, P], F8, tag="wv8lo")
            a8_s = pers.tile([P, 6, CH], F8, tag="a8")
            evwb_s = pers.tile([P, J], BF, tag="evwb")
            bke_s = pers.tile([1, J], BF, tag="bke")
            wout_s = pers.tile([P, KB, NCLS], BF, tag="wout")
            bsk_s = pers.tile([P, 1], F32, tag="bsk")
            bv_s = pers.tile([P, CHB], F32, tag="bv")
            bout_s = pers.tile([NCLS, 1], F32, tag="bout")
            ident_s = pers.tile([P, P], BF, tag="ident")
            ones_s = pers.tile([P, 1], BF, tag="ones")
            ones1_s = pers.tile([1, P], BF, tag="ones1")
            scratch_s = pers.tile([1, 1], F32, tag="scratch")
            sk_s = pers.tile([P, BL], BF, tag="sk")
            sksq_s = pers.tile([P, BL], BF, tag="sksq")
            hv_s = pers.tile([P, CHB, BL], BF, tag="hv")
            hfe_s = pers.tile([P, CHB, BL], BF, tag="hfe")
            e_s = pers.tile([P, NB, J], BF, tag="e")
            tmp_s = pers.tile([P, J], BF, tag="tmp")
            u_s = pers.tile([P, NB, J], BF, tag="u")
            ut_s = pers.tile([P, 6, BL], F8, tag="ut")
            rinv_s = pers.tile([P, NB], F32, tag="rinv")
            rs1_s = pers.tile([P, NB], F32, tag="rs1")
            rs2_s = pers.tile([P, NB], F32, tag="rs2")
            magic_s = pers.tile([P, 1], mybir.dt.int32, tag="magic")
            s_s = pers.tile([P, NB * NCLS], BF, tag="s")
            num_s = pers.tile([P, NB * NCLS], BF, tag="num")
            sinv_s = pers.tile([P, NB * NCLS], F32, tag="sinv")
            t_s = pers.tile([P, NB * NCLS], F32, tag="t")
            g_s = pers.tile([P, NB * NCLS], F32, tag="g")
            gg_s = pers.tile([P, NB], F32, tag="gg")
            ginv_s = pers.tile([P, NB], F32, tag="ginv")
            c1_s = pers.tile([P, NB * NCLS], F32, tag="c1")
            c_s = pers.tile([P, NB * NCLS], F32, tag="c")
            out_sb = pers.tile([NCLS, BL], F32, tag="outsb")

            # ---- setup: memsets + pin the Exp ACT table before any evict ----
            nc.vector.memset(ones_s[:], 1.0)
            nc.vector.memset(ones1_s[:], 1.0)
            nc.vector.memset(magic_s[:], 0x5f3759df)
            nc.vector.memset(ut_s[:], 0.0)        # zero jb-5 pad partitions
            nc.vector.memset(scratch_s[:], 1.0)
            nc.scalar.activation(scratch_s[:], scratch_s[:], AF.Exp)

            # ---- DMA schedule (sync queue; DMA device is the critical
            # resource at ~22.6us busy). wv pairs last: each gates only one
            # v pass. ----
            nc.sync.dma_start(x8_s[:], x8_e.ap())
            nc.sync.dma_start(wsk8_s[:], wsk8_e.ap())
            nc.sync.dma_start(bke_s[:], bke_e.ap())
            nc.sync.dma_start(evwb_s[:], evwb_e.ap())
            nc.sync.dma_start(d8_s[:], d8_e.ap())
            nc.sync.dma_start(x8lo_s[:], x8lo_e.ap())
            nc.sync.dma_start(wout_s[:], wout_e.ap())
            for o in range(CHB):
                nc.sync.dma_start(wv8_s[:, o, :, :], wv8_e.ap()[o])
                nc.sync.dma_start(wv8lo_s[:, o, :, :], wv8lo_e.ap()[o])
                if o == 1:
                    nc.sync.dma_start(a8_s[:], a8_e.ap())
            nc.gpsimd.dma_start(bsk_s[:], bsk_e.ap())
            nc.gpsimd.dma_start(bv_s[:], bv_e.ap())
            nc.gpsimd.dma_start(bout_s[:], bout_e.ap())
            nc.gpsimd.dma_start(ident_s[:], ident_e.ap())

            # ---- phase 1: norm sketch: sk = 64*S*k, rinv = 1/||sk|| ----
            ps = pkv.tile([P, BL], F32, tag="pkv")
            for p in range(KP):
                nc.tensor.matmul(ps[0:RSK, :], wsk8_s[:, 2 * p:2 * p + 2, :],
                                 x8_s[:, 2 * p:2 * p + 2, :],
                                 start=(p == 0), stop=(p == KP - 1),
                                 perf_mode=DR)
            nc.scalar.activation(sk_s[0:RSK, :], ps[0:RSK, :], AF.Identity,
                                 bias=bsk_s[0:RSK, :])
            nc.vector.tensor_mul(sksq_s[0:RSK, :], sk_s[0:RSK, :],
                                 sk_s[0:RSK, :])
            ps2 = pmisc.tile([P, NB], F32, tag="misc")
            for bc in range(NB):
                nc.tensor.matmul(ps2[:, bc:bc + 1],
                                 sksq_s[0:RSK, bc * P:(bc + 1) * P],
                                 ones_s[0:RSK, :],
                                 start=True, stop=True)
                sq = rs1_s[:, bc:bc + 1]
                nc.vector.tensor_copy(sq, ps2[:, bc:bc + 1])
                y = rinv_s[:, bc:bc + 1]
                nc.vector.tensor_scalar(
                    y.bitcast(mybir.dt.int32), sq.bitcast(mybir.dt.int32),
                    1, None, op0=mybir.AluOpType.logical_shift_right)
                nc.vector.tensor_tensor(
                    out=y.bitcast(mybir.dt.int32), in0=magic_s[:],
                    in1=y.bitcast(mybir.dt.int32),
                    op=mybir.AluOpType.subtract)
                for _ in range(2):
                    t1 = rs2_s[:, bc:bc + 1]
                    nc.vector.tensor_mul(t1, y, y)
                    nc.vector.tensor_mul(t1, t1, sq)
                    nc.vector.tensor_scalar(t1, t1, -0.5, 1.5,
                                            op0=mybir.AluOpType.mult,
                                            op1=mybir.AluOpType.add)
                    nc.vector.tensor_mul(y, y, t1)

            # ---- phase 2: dots = x8 @ D8 (+bkE), chunk-major fp8 DR ----
            def dots(bc):
                psd = pdot.tile([P, J], F32, tag="pdot")
                for (c0, c1) in JC:
                    for p in range(KP):
                        nc.tensor.matmul(
                            psd[:, c0:c1],
                            x8_s[:, 2 * p:2 * p + 2, bc * P:bc * P + P],
                            d8_s[:, 2 * p:2 * p + 2, c0:c1],
                            start=(p == 0), stop=False, perf_mode=DR)
                    # += bkE (K=1 rank-1 broadcast matmul closes the group)
                    nc.tensor.matmul(psd[:, c0:c1], ones1_s[:],
                                     bke_s[:, c0:c1], start=False, stop=True)
                nc.scalar.activation(e_s[:, bc, 0:512], psd[:, 0:512], AF.Exp,
                                     scale=rinv_s[:, bc:bc + 1])
                nc.scalar.activation(e_s[:, bc, 512:J], psd[:, 512:J], AF.Exp,
                                     scale=rinv_s[:, bc:bc + 1])

            def softmax_chain(bc, eng):
                lp = nc.allow_low_precision(
                    reason="S/num feed softmax ratios; errors attenuate")
                lp.__enter__()
                # bc0 runs on DVE, bc1 on gpsimd: the two chains execute in
                # parallel so u1 lands ~2us earlier
                e_sl = e_s[:, bc, :]
                e3 = e_sl.rearrange("p (n m) -> p n m", m=M)
                ncls_sl = slice(bc * NCLS, (bc + 1) * NCLS)
                s2 = s_s[:, ncls_sl]
                eng.reduce_sum(s2, e3, axis=mybir.AxisListType.X)
                u_tmp = u_s[:, bc, :]
                eng.tensor_mul(u_tmp, e_sl, evwb_s[:])
                eng.reduce_sum(num_s[:, ncls_sl],
                               u_tmp.rearrange("p (n m) -> p n m", m=M),
                               axis=mybir.AxisListType.X)
                eng.reciprocal(sinv_s[:, ncls_sl], s2)
                eng.tensor_mul(t_s[:, ncls_sl], num_s[:, ncls_sl],
                               sinv_s[:, ncls_sl])
                nc.scalar.activation(g_s[:, ncls_sl], t_s[:, ncls_sl], AF.Exp)
                eng.reduce_sum(gg_s[:, bc:bc + 1], g_s[:, ncls_sl],
                               axis=mybir.AxisListType.X)
                eng.reciprocal(ginv_s[:, bc:bc + 1], gg_s[:, bc:bc + 1])
                # fold the u scale SU into ginv: c = g*sinv * (SU/G)
                eng.tensor_scalar(ginv_s[:, bc:bc + 1], ginv_s[:, bc:bc + 1],
                                  SU, None, op0=mybir.AluOpType.mult)
                eng.tensor_mul(c1_s[:, ncls_sl], g_s[:, ncls_sl],
                               sinv_s[:, ncls_sl])
                eng.tensor_scalar_mul(c_s[:, ncls_sl], c1_s[:, ncls_sl],
                                      ginv_s[:, bc:bc + 1])
                c_b = bass.AP(c_s.tensor, c_s[:, ncls_sl].offset,
                              c_s[:, ncls_sl].ap + [[0, M]])
                u3 = u_s[:, bc, :].rearrange("p (n m) -> p n m", m=M)
                eng.tensor_mul(u3, e3, c_b)
                lp.__exit__(None, None, None)

            # ---- out accumulator: block matmuls emitted as inputs land ----
            pso = pmisc.tile([NCLS, BL], F32, tag="misc")
            n_out_mm = [0]

            def out_mm(h_s, i):
                kb = i if h_s is hv_s else CHB + i
                nc.tensor.matmul(pso[:], wout_s[:, kb, :], h_s[:, i, :],
                                 start=(n_out_mm[0] == 0),
                                 stop=(n_out_mm[0] == KB - 1))
                n_out_mm[0] += 1

            def v_block(o):
                ps = pkv.tile([P, BL], F32, tag="pkv")
                n = 0
                for (wt, xt) in ((wv8_s, x8_s), (wv8_s, x8lo_s),
                                 (wv8lo_s, x8_s)):
                    for p in range(KP):
                        nc.tensor.matmul(ps[:], wt[:, o, 2 * p:2 * p + 2, :],
                                         xt[:, 2 * p:2 * p + 2, :],
                                         start=(n == 0), stop=(n == 3 * KP - 1),
                                         perf_mode=DR)
                        n += 1
                nc.scalar.activation(hv_s[:, o, :], ps[:], AF.Relu,
                                     scale=1.0 / SW, bias=bv_s[:, o:o + 1])

            def transpose_u(bc):
                def tgroup(grp, pool, ptag):
                    pst = pool.tile([P, 3 * P], BF, tag=ptag)
                    for t, jb in enumerate(grp):
                        w = P if jb < 5 else J - 5 * P
                        nc.tensor.transpose(
                            pst[:w, t * P:(t + 1) * P],
                            u_s[:, bc, jb * P:jb * P + w],
                            ident_s[:])
                    n = sum(1 for jb in grp if jb < 5)
                    base = ut_s[:, grp[0], bc * P:bc * P + P]
                    dst = bass.AP(ut_s.tensor, base.offset,
                                  [base.ap[0], [BL, n], base.ap[1]])
                    nc.vector.tensor_copy(
                        dst, pst[:, 0:n * P].rearrange("p (n q) -> p n q", q=P))
                    if n < len(grp):
                        jb = grp[n]
                        w = J - 5 * P
                        nc.scalar.activation(
                            ut_s[:w, jb, bc * P:bc * P + P],
                            pst[:w, n * P:(n + 1) * P], AF.Identity)
                # alternate psum pools so the four transpose groups pipeline
                tgroup((0, 1, 2), ptr, "ptr")
                tgroup((3, 4, 5), pfe, f"pfe{bc}")

            def fe_all():
                # rotate through three psum slots (pdot is idle after exps)
                for o in range(CHB):
                    if o % 3 < 2:
                        acc = pfe.tile([P, BL], F32, tag=f"pfe{o % 3}")
                    else:
                        acc = pdot.tile([P, BL], F32, tag="pdot")
                    for t in range(3):
                        nc.tensor.matmul(acc[:], a8_s[:, 2 * t:2 * t + 2,
                                                      o * P:(o + 1) * P],
                                         ut_s[:, 2 * t:2 * t + 2, :],
                                         start=(t == 0), stop=(t == 2),
                                         perf_mode=DR)
                    dst = hfe_s[:, o, :]
                    if o % 2 == 0:
                        nc.scalar.activation(dst, acc[:], AF.Relu,
                                             scale=1.0 / (SA * SU))
                    else:
                        nc.vector.tensor_scalar(dst, acc[:],
                                                1.0 / (SA * SU), 0.0,
                                                op0=mybir.AluOpType.mult,
                                                op1=mybir.AluOpType.max)

            # ---- main interleave: the whole dots/softmax/transpose/fE chain
            # runs before the wv stream thickens; v blocks then track DMA ----
            dots(0)
            softmax_chain(0, nc.vector)
            dots(1)
            softmax_chain(1, nc.vector)
            v_block(0)
            v_block(1)
            out_mm(hv_s, 0)
            transpose_u(0)
            transpose_u(1)
            fe_all()
            for i in range(CHB):
                out_mm(hfe_s, i)
            for o in range(2, CHB):
                v_block(o)
                out_mm(hv_s, o - 1)
            out_mm(hv_s, CHB - 1)

            # ---- +bout, DMA out ----
            nc.vector.tensor_scalar_add(out_sb[:], pso[:], bout_s[:, 0:1])
            nc.sync.dma_start(out_e.ap(), out_sb[:])

    nc.compile()
    return nc


def host_prep(x, static_feat, Wk, bk, Wv, bv, WEk, bEk, WEv, bEv, Ww, bw,
              Wout, bout):
    """Host-side fp32 precompute, fp8/bf16 quantization, per-core input maps."""
    EPS = 1e-8
    f32 = np.float32
    x = np.asarray(x, f32)
    static_feat = np.asarray(static_feat, f32)
    Wk, bk = np.asarray(Wk, f32), np.asarray(bk, f32)
    Wv, bv = np.asarray(Wv, f32), np.asarray(bv, f32)
    Wout, bout = np.asarray(Wout, f32), np.asarray(bout, f32)

    Ek = np.einsum('oc,ncm->nom', np.asarray(WEk, f32), static_feat,
                   optimize=True) + np.asarray(bEk, f32)[None, :, None]
    Ev = np.einsum('oc,ncm->nom', np.asarray(WEv, f32), static_feat,
                   optimize=True) + np.asarray(bEv, f32)[None, :, None]
    Ekn = Ek / np.maximum(np.linalg.norm(Ek, axis=1, keepdims=True), EPS)
    Ekn_mat = Ekn.transpose(1, 0, 2).reshape(CH, J)          # [CH, 672]
    A_mat = Ev.transpose(0, 2, 1).reshape(J, CH)             # [672, CH]
    evwb = np.einsum('nom,o->nm', Ev, np.asarray(Ww, f32)[0]).reshape(J)

    # norm sketch + folded dots
    S = np.random.RandomState(0).randn(RSK, CH).astype(f32) / np.sqrt(RSK)
    W_sk = S @ Wk                                            # [128, CIN]
    b_sk = S @ bk
    D = Wk.T @ Ekn_mat                                       # [CIN, J]
    bkE = bk @ Ekn_mat                                       # [J]

    def cinlayout(w, width):    # [CIN, width] -> [P, KB*width]
        return np.ascontiguousarray(
            w.reshape(KB, P, width).transpose(1, 0, 2).reshape(P, KB * width))

    wsk8_h = cinlayout((W_sk.T * SD).astype(f8), RSK)
    d8_h = cinlayout((D * SD).astype(f8), J)

    def wlayout(w):     # [CIN, OCH] f8 -> dram [OCH/P, P, KB*P]
        och = w.shape[1]
        return np.ascontiguousarray(
            w.reshape(KB, P, och // P, P).transpose(2, 1, 0, 3)
            .reshape(och // P, P, KB * P))

    wv_s = Wv.T * SW
    wv8_f = wv_s.astype(f8)
    wv8lo_f = (wv_s - wv8_f.astype(f32)).astype(f8)
    wv8_h = wlayout(wv8_f)
    wv8lo_h = wlayout(wv8lo_f)

    a_pad = np.zeros((6 * P, CH), f32)
    a_pad[:J] = A_mat * SA
    a8_h = np.ascontiguousarray(
        a_pad.astype(f8).reshape(6, P, CH).transpose(1, 0, 2).reshape(P, 6 * CH))
    evwb_h = np.ascontiguousarray(
        np.broadcast_to(evwb.astype(bf16)[None, :], (P, J)))
    bke_h = (bkE * SD).astype(bf16).reshape(1, J)
    wout_h = np.ascontiguousarray(
        Wout.T.reshape(KB, P, NCLS).transpose(1, 0, 2).reshape(
            P, KB * NCLS).astype(bf16))
    bsk_h = np.zeros((P, 1), f32)
    bsk_h[:RSK, 0] = b_sk * SD
    bv_h = np.ascontiguousarray(bv.reshape(CHB, P).T)
    bout_h = bout.reshape(NCLS, 1)
    ident_h = np.eye(P, dtype=bf16)

    xT = np.ascontiguousarray(x[:, -1, :].T)                 # [CIN, B]
    x8_f = xT.astype(f8)
    x8lo_f = (xT - x8_f.astype(f32)).astype(f8)

    shared = dict(wsk8=wsk8_h, d8=d8_h, wv8=wv8_h, wv8lo=wv8lo_h, a8=a8_h,
                  evwb=evwb_h, bke=bke_h, wout=wout_h, bsk=bsk_h, bv=bv_h,
                  bout=bout_h, ident=ident_h)
    in_maps = []
    for c in range(NCORES):
        sl = slice(c * BL, (c + 1) * BL)

        def xlayout(xf):
            return np.ascontiguousarray(
                xf[:, sl].reshape(KB, P, BL).transpose(1, 0, 2)
                .reshape(P, KB * BL))
        in_maps.append(dict(x8=xlayout(x8_f), x8lo=xlayout(x8lo_f), **shared))
    return in_maps


_NC_CACHE = {}


def get_nc(debug=False):
    if debug not in _NC_CACHE:
        _NC_CACHE[debug] = build_nc(debug=debug)
    return _NC_CACHE[debug]


def kernel(**inputs) -> np.ndarray:
    nc = get_nc()
    in_maps = host_prep(**inputs)
    res = run_bass_kernel_spmd(nc, in_maps, list(range(NCORES)))
    out = np.empty((B, NCLS, 1), dtype=np.float32)
    for c in range(NCORES):
        out[c * BL:(c + 1) * BL, :, 0] = res.results[c]["out"].T
    return out


# revision 40
# speedup vs baseline: 1.0198x; 1.0022x over previous
"""Trainium2 Bass kernel for nn_Colar_static (retrieval_knn).

Sharding: data-parallel over batch B=2048 across 8 NeuronCores (256 rows each).
Weights/exemplars replicated, precomputed + quantized on host.

Design (vs the bf16 baseline at 53.3us):
  * Every large matmul is fp8e4m3 with the DoubleRow perf mode (K=256 per
    instruction, 0.5 cycles/row) -> 4x bf16 MAC rate and 1-byte weights
    (the kernel is DMA-bound: all DMA serializes at ~332 GB/s).
  * dots = x @ (Wk^T Ekn) directly: Wk is folded into the exemplars on the
    host, so the 2MB Wk and 0.7MB Ekn never ship; only D8 [CIN,672] (1.4MB).
  * ||k|| (softmax temperature only) via a random sketch: ||S k|| with
    S [128,1024] Gaussian, W_sk = S Wk [128, CIN] fp8 (0.25MB). The 5% norm
    error is invisible downstream (validated: rel err 3.21e-3, same as the
    exact-norm pipeline, because cos logits are tiny and softmax-smoothed).
  * v  = x8@Wv8hi + x8lo@Wv8hi + x8@Wv8lo   3-pass residual-compensated fp8
    (v dominates the output; plain fp8 fails at 3.3e-2).
  * fE = A8^T @ ut8 (fp8 DR);  out = Wout^T @ [hv;hfe] in bf16 (tiny).

Scales (all folded, no extra device work): D,W_sk x64; Wv x32; A x16; u x256.
The sketch scale cancels: rinv = rsqrt(sum((64 S k)^2)) = 1/(64||Sk||) and
dots are x64, so exp(dots*rinv) = exp(cos).

Rel err vs fp32 reference ~3.2e-3 (numpy-sim validated; gate is 2e-2).
"""

import numpy as np
import ml_dtypes

import concourse.bass as bass
import concourse.bacc as bacc
import concourse.mybir as mybir
import concourse.tile as tile
from concourse.bass_utils import run_bass_kernel_spmd

AF = mybir.ActivationFunctionType
BF = mybir.dt.bfloat16
F8 = mybir.dt.float8e4
F32 = mybir.dt.float32
DR = mybir.MatmulPerfMode.DoubleRow
bf16 = ml_dtypes.bfloat16
f8 = ml_dtypes.float8_e4m3

# Problem constants (hardcoded; kernel.py must be self-contained)
B, T, CIN, CH, M, NCLS = 2048, 8, 2048, 1024, 32, 21
NCORES = 8
BL = B // NCORES          # 256 batch rows per core
J = NCLS * M              # 672
P = 128
KB = CIN // P             # 16 contraction blocks over CIN
KP = KB // 2              # 8 DoubleRow pairs over CIN
CHB = CH // P             # 8 blocks over CH
NB = BL // P              # 2 batch chunks of 128
RSK = 64                  # norm-sketch rank
SD, SW, SA, SU = 64.0, 32.0, 16.0, 256.0
JC = [(0, 256), (256, 512), (512, J)]   # dots psum chunks (bank-safe)


def build_nc(debug=False):
    nc = bacc.Bacc("TRN2", target_bir_lowering=False, debug=debug,
                   num_devices=NCORES)

    x8_e = nc.dram_tensor("x8", [P, KB * BL], F8, kind="ExternalInput")
    x8lo_e = nc.dram_tensor("x8lo", [P, KB * BL], F8, kind="ExternalInput")
    wsk8_e = nc.dram_tensor("wsk8", [P, KB * RSK], F8, kind="ExternalInput")
    d8_e = nc.dram_tensor("d8", [P, KB * J], F8, kind="ExternalInput")
    wv8_e = nc.dram_tensor("wv8", [CHB, P, KB * P], F8, kind="ExternalInput")
    wv8lo_e = nc.dram_tensor("wv8lo", [CHB, P, KB * P], F8, kind="ExternalInput")
    a8_e = nc.dram_tensor("a8", [P, 6 * CH], F8, kind="ExternalInput")
    evwb_e = nc.dram_tensor("evwb", [P, J], BF, kind="ExternalInput")
    bke_e = nc.dram_tensor("bke", [1, J], BF, kind="ExternalInput")
    wout_e = nc.dram_tensor("wout", [P, KB * NCLS], BF, kind="ExternalInput")
    bsk_e = nc.dram_tensor("bsk", [P, 1], F32, kind="ExternalInput")
    bv_e = nc.dram_tensor("bv", [P, 2 * CHB], F32, kind="ExternalInput")
    bout_e = nc.dram_tensor("bout", [NCLS, 1], F32, kind="ExternalInput")
    ident_e = nc.dram_tensor("ident", [P, P], BF, kind="ExternalInput")
    out_e = nc.dram_tensor("out", [NCLS, BL], F32, kind="ExternalOutput")

    with tile.TileContext(nc) as tc:
        from contextlib import ExitStack
        with ExitStack() as ctx:
            pers = ctx.enter_context(tc.tile_pool(name="pers", bufs=1))
            pmisc = ctx.enter_context(tc.tile_pool(name="pmisc", bufs=1, space="PSUM"))
            pkv = ctx.enter_context(tc.tile_pool(name="pkv", bufs=2, space="PSUM"))
            pdot = ctx.enter_context(tc.tile_pool(name="pdot", bufs=1, space="PSUM"))
            ptr = ctx.enter_context(tc.tile_pool(name="ptr", bufs=1, space="PSUM"))
            pfe = ctx.enter_context(tc.tile_pool(name="pfe", bufs=1, space="PSUM"))

            # ---- SBUF tiles ----
            x8_s = pers.tile([P, KB, BL], F8, tag="x8")
            x8lo_s = pers.tile([P, KB, BL], F8, tag="x8lo")
            wsk8_s = pers.tile([P, KB, RSK], F8, tag="wsk8")
            d8_s = pers.tile([P, KB, J], F8, tag="d8")
            wv8_s = pers.tile([P, CHB, KB, P], F8, tag="wv8")
            wv8lo_s = pers.tile([P, CHB, KB, P], F8, tag="wv8lo")
            a8_s = pers.tile([P, 6, CH], F8, tag="a8")
            evwb_s = pers.tile([P, J], BF, tag="evwb")
            bke_s = pers.tile([1, J], BF, tag="bke")
            wout_s = pers.tile([P, KB, NCLS], BF, tag="wout")
            bsk_s = pers.tile([P, 1], F32, tag="bsk")
            bv_s = pers.tile([P, 2 * CHB], F32, tag="bv")
            bout_s = pers.tile([NCLS, 1], F32, tag="bout")
            ident_s = pers.tile([P, P], BF, tag="ident")
            ones_s = pers.tile([P, 1], BF, tag="ones")
            ones1_s = pers.tile([1, P], BF, tag="ones1")
            scratch_s = pers.tile([1, 1], F32, tag="scratch")
            sk_s = pers.tile([P, BL], BF, tag="sk")
            sksq_s = pers.tile([P, BL], BF, tag="sksq")
            hv_s = pers.tile([P, CHB, BL], BF, tag="hv")
            hfe_s = pers.tile([P, CHB, BL], BF, tag="hfe")
            e_s = pers.tile([P, NB, J], BF, tag="e")
            tmp_s = pers.tile([P, J], BF, tag="tmp")
            u_s = pers.tile([P, NB, J], BF, tag="u")
            ut_s = pers.tile([P, 6, BL], F8, tag="ut")
            rinv_s = pers.tile([P, NB], F32, tag="rinv")
            rs1_s = pers.tile([P, NB], F32, tag="rs1")
            rs2_s = pers.tile([P, NB], F32, tag="rs2")
            magic_s = pers.tile([P, 1], mybir.dt.int32, tag="magic")
            s_s = pers.tile([P, NB * NCLS], BF, tag="s")
            num_s = pers.tile([P, NB * NCLS], BF, tag="num")
            sinv_s = pers.tile([P, NB * NCLS], F32, tag="sinv")
            t_s = pers.tile([P, NB * NCLS], F32, tag="t")
            g_s = pers.tile([P, NB * NCLS], F32, tag="g")
            gg_s = pers.tile([P, NB], F32, tag="gg")
            ginv_s = pers.tile([P, NB], F32, tag="ginv")
            c1_s = pers.tile([P, NB * NCLS], F32, tag="c1")
            c_s = pers.tile([P, NB * NCLS], F32, tag="c")
            out_sb = pers.tile([NCLS, BL], F32, tag="outsb")

            # ---- setup: memsets + pin the Exp ACT table before any evict ----
            nc.vector.memset(ones_s[:], 1.0)
            nc.vector.memset(ones1_s[:], 1.0)
            nc.vector.memset(magic_s[:], 0x5f3759df)
            nc.vector.memset(ut_s[:], 0.0)        # zero jb-5 pad partitions
            nc.vector.memset(scratch_s[:], 1.0)
            nc.scalar.activation(scratch_s[:], scratch_s[:], AF.Exp)

            # ---- DMA schedule (sync queue; DMA device is the critical
            # resource at ~22.6us busy). wv pairs last: each gates only one
            # v pass. ----
            nc.sync.dma_start(x8_s[:], x8_e.ap())
            nc.sync.dma_start(wsk8_s[:], wsk8_e.ap())
            nc.sync.dma_start(bke_s[:], bke_e.ap())
            nc.sync.dma_start(evwb_s[:], evwb_e.ap())
            nc.sync.dma_start(d8_s[:], d8_e.ap())
            nc.sync.dma_start(x8lo_s[:], x8lo_e.ap())
            nc.sync.dma_start(wout_s[:], wout_e.ap())
            for o in range(CHB):
                nc.sync.dma_start(wv8_s[:, o, :, :], wv8_e.ap()[o])
                nc.sync.dma_start(wv8lo_s[:, o, :, :], wv8lo_e.ap()[o])
                if o == 1:
                    nc.sync.dma_start(a8_s[:], a8_e.ap())
            nc.gpsimd.dma_start(bsk_s[:], bsk_e.ap())
            nc.gpsimd.dma_start(bv_s[:], bv_e.ap())
            nc.gpsimd.dma_start(bout_s[:], bout_e.ap())
            nc.gpsimd.dma_start(ident_s[:], ident_e.ap())

            # ---- phase 1: norm sketch: sk = 64*S*k, rinv = 1/||sk|| ----
            ps = pkv.tile([P, BL], F32, tag="pkv")
            for p in range(KP):
                nc.tensor.matmul(ps[0:RSK, :], wsk8_s[:, 2 * p:2 * p + 2, :],
                                 x8_s[:, 2 * p:2 * p + 2, :],
                                 start=(p == 0), stop=(p == KP - 1),
                                 perf_mode=DR)
            nc.scalar.activation(sk_s[0:RSK, :], ps[0:RSK, :], AF.Identity,
                                 bias=bsk_s[0:RSK, :])
            nc.vector.tensor_mul(sksq_s[0:RSK, :], sk_s[0:RSK, :],
                                 sk_s[0:RSK, :])
            ps2 = pmisc.tile([P, NB], F32, tag="misc")
            for bc in range(NB):
                nc.tensor.matmul(ps2[:, bc:bc + 1],
                                 sksq_s[0:RSK, bc * P:(bc + 1) * P],
                                 ones_s[0:RSK, :],
                                 start=True, stop=True)
                sq = rs1_s[:, bc:bc + 1]
                nc.vector.tensor_copy(sq, ps2[:, bc:bc + 1])
                y = rinv_s[:, bc:bc + 1]
                nc.vector.tensor_scalar(
                    y.bitcast(mybir.dt.int32), sq.bitcast(mybir.dt.int32),
                    1, None, op0=mybir.AluOpType.logical_shift_right)
                nc.vector.tensor_tensor(
                    out=y.bitcast(mybir.dt.int32), in0=magic_s[:],
                    in1=y.bitcast(mybir.dt.int32),
                    op=mybir.AluOpType.subtract)
                for _ in range(2):
                    t1 = rs2_s[:, bc:bc + 1]
                    nc.vector.tensor_mul(t1, y, y)
                    nc.vector.tensor_mul(t1, t1, sq)
                    nc.vector.tensor_scalar(t1, t1, -0.5, 1.5,
                                            op0=mybir.AluOpType.mult,
                                            op1=mybir.AluOpType.add)
                    nc.vector.tensor_mul(y, y, t1)

            # ---- phase 2: dots = x8 @ D8 (+bkE), chunk-major fp8 DR ----
            def dots(bc):
                psd = pdot.tile([P, J], F32, tag="pdot")
                for (c0, c1) in JC:
                    for p in range(KP):
                        nc.tensor.matmul(
                            psd[:, c0:c1],
                            x8_s[:, 2 * p:2 * p + 2, bc * P:bc * P + P],
                            d8_s[:, 2 * p:2 * p + 2, c0:c1],
                            start=(p == 0), stop=False, perf_mode=DR)
                    # += bkE (K=1 rank-1 broadcast matmul closes the group)
                    nc.tensor.matmul(psd[:, c0:c1], ones1_s[:],
                                     bke_s[:, c0:c1], start=False, stop=True)
                nc.scalar.activation(e_s[:, bc, 0:512], psd[:, 0:512], AF.Exp,
                                     scale=rinv_s[:, bc:bc + 1])
                nc.scalar.activation(e_s[:, bc, 512:J], psd[:, 512:J], AF.Exp,
                                     scale=rinv_s[:, bc:bc + 1])

            def softmax_chain(bc, eng):
                lp = nc.allow_low_precision(
                    reason="S/num feed softmax ratios; errors attenuate")
                lp.__enter__()
                # bc0 runs on DVE, bc1 on gpsimd: the two chains execute in
                # parallel so u1 lands ~2us earlier
                e_sl = e_s[:, bc, :]
                e3 = e_sl.rearrange("p (n m) -> p n m", m=M)
                ncls_sl = slice(bc * NCLS, (bc + 1) * NCLS)
                s2 = s_s[:, ncls_sl]
                eng.reduce_sum(s2, e3, axis=mybir.AxisListType.X)
                u_tmp = u_s[:, bc, :]
                eng.tensor_mul(u_tmp, e_sl, evwb_s[:])
                eng.reduce_sum(num_s[:, ncls_sl],
                               u_tmp.rearrange("p (n m) -> p n m", m=M),
                               axis=mybir.AxisListType.X)
                eng.reciprocal(sinv_s[:, ncls_sl], s2)
                eng.tensor_mul(t_s[:, ncls_sl], num_s[:, ncls_sl],
                               sinv_s[:, ncls_sl])
                nc.scalar.activation(g_s[:, ncls_sl], t_s[:, ncls_sl], AF.Exp)
                eng.reduce_sum(gg_s[:, bc:bc + 1], g_s[:, ncls_sl],
                               axis=mybir.AxisListType.X)
                eng.reciprocal(ginv_s[:, bc:bc + 1], gg_s[:, bc:bc + 1])
                # fold the u scale SU into ginv: c = g*sinv * (SU/G)
                eng.tensor_scalar(ginv_s[:, bc:bc + 1], ginv_s[:, bc:bc + 1],
                                  SU, None, op0=mybir.AluOpType.mult)
                eng.tensor_mul(c1_s[:, ncls_sl], g_s[:, ncls_sl],
                               sinv_s[:, ncls_sl])
                eng.tensor_scalar_mul(c_s[:, ncls_sl], c1_s[:, ncls_sl],
                                      ginv_s[:, bc:bc + 1])
                c_b = bass.AP(c_s.tensor, c_s[:, ncls_sl].offset,
                              c_s[:, ncls_sl].ap + [[0, M]])
                u3 = u_s[:, bc, :].rearrange("p (n m) -> p n m", m=M)
                eng.tensor_mul(u3, e3, c_b)
                lp.__exit__(None, None, None)

            # ---- out accumulator: block matmuls emitted as inputs land ----
            pso = pmisc.tile([NCLS, BL], F32, tag="misc")
            n_out_mm = [0]

            def out_mm(h_s, i):
                kb = i if h_s is hv_s else CHB + i
                nc.tensor.matmul(pso[:], wout_s[:, kb, :], h_s[:, i, :],
                                 start=(n_out_mm[0] == 0),
                                 stop=(n_out_mm[0] == KB - 1))
                n_out_mm[0] += 1

            def v_block(o):
                ps = pkv.tile([P, BL], F32, tag="pkv")
                n = 0
                for (wt, xt) in ((wv8_s, x8_s), (wv8_s, x8lo_s),
                                 (wv8lo_s, x8_s)):
                    for p in range(KP):
                        nc.tensor.matmul(ps[:], wt[:, o, 2 * p:2 * p + 2, :],
                                         xt[:, 2 * p:2 * p + 2, :],
                                         start=(n == 0), stop=(n == 3 * KP - 1),
                                         perf_mode=DR)
                        n += 1
                if o % 2 == 0:
                    nc.scalar.activation(hv_s[:, o, :], ps[:], AF.Relu,
                                         scale=1.0 / SW, bias=bv_s[:, o:o + 1])
                else:
                    # DVE path: max(psum + 32*bv, 0) = 32*hv; the 1/32 is
                    # folded into this block's wout column on the host
                    nc.vector.tensor_scalar(hv_s[:, o, :], ps[:],
                                            bv_s[:, CHB + o:CHB + o + 1], 0.0,
                                            op0=mybir.AluOpType.add,
                                            op1=mybir.AluOpType.max)

            def transpose_u(bc):
                def tgroup(grp, pool, ptag):
                    pst = pool.tile([P, 3 * P], BF, tag=ptag)
                    for t, jb in enumerate(grp):
                        w = P if jb < 5 else J - 5 * P
                        nc.tensor.transpose(
                            pst[:w, t * P:(t + 1) * P],
                            u_s[:, bc, jb * P:jb * P + w],
                            ident_s[:])
                    n = sum(1 for jb in grp if jb < 5)
                    base = ut_s[:, grp[0], bc * P:bc * P + P]
                    dst = bass.AP(ut_s.tensor, base.offset,
                                  [base.ap[0], [BL, n], base.ap[1]])
                    nc.vector.tensor_copy(
                        dst, pst[:, 0:n * P].rearrange("p (n q) -> p n q", q=P))
                    if n < len(grp):
                        jb = grp[n]
                        w = J - 5 * P
                        nc.scalar.activation(
                            ut_s[:w, jb, bc * P:bc * P + P],
                            pst[:w, n * P:(n + 1) * P], AF.Identity)
                # alternate psum pools so the four transpose groups pipeline
                tgroup((0, 1, 2), ptr, "ptr")
                tgroup((3, 4, 5), pfe, f"pfe{bc}")

            def fe_all():
                # rotate through three psum slots (pdot is idle after exps)
                for o in range(CHB):
                    if o % 3 < 2:
                        acc = pfe.tile([P, BL], F32, tag=f"pfe{o % 3}")
                    else:
                        acc = pdot.tile([P, BL], F32, tag="pdot")
                    for t in range(3):
                        nc.tensor.matmul(acc[:], a8_s[:, 2 * t:2 * t + 2,
                                                      o * P:(o + 1) * P],
                                         ut_s[:, 2 * t:2 * t + 2, :],
                                         start=(t == 0), stop=(t == 2),
                                         perf_mode=DR)
                    dst = hfe_s[:, o, :]
                    if o % 2 == 0:
                        nc.scalar.activation(dst, acc[:], AF.Relu,
                                             scale=1.0 / (SA * SU))
                    else:
                        nc.vector.tensor_scalar(dst, acc[:],
                                                1.0 / (SA * SU), 0.0,
                                                op0=mybir.AluOpType.mult,
                                                op1=mybir.AluOpType.max)

            # ---- main interleave: the whole dots/softmax/transpose/fE chain
            # runs before the wv stream thickens; v blocks then track DMA ----
            dots(0)
            softmax_chain(0, nc.vector)
            dots(1)
            softmax_chain(1, nc.vector)
            v_block(0)
            v_block(1)
            out_mm(hv_s, 0)
            transpose_u(0)
            transpose_u(1)
            fe_all()
            for i in range(CHB):
                out_mm(hfe_s, i)
            for o in range(2, CHB):
                v_block(o)
                out_mm(hv_s, o - 1)
            out_mm(hv_s, CHB - 1)

            # ---- +bout, DMA out ----
            nc.vector.tensor_scalar_add(out_sb[:], pso[:], bout_s[:, 0:1])
            nc.sync.dma_start(out_e.ap(), out_sb[:])

    nc.compile()
    return nc


def host_prep(x, static_feat, Wk, bk, Wv, bv, WEk, bEk, WEv, bEv, Ww, bw,
              Wout, bout):
    """Host-side fp32 precompute, fp8/bf16 quantization, per-core input maps."""
    EPS = 1e-8
    f32 = np.float32
    x = np.asarray(x, f32)
    static_feat = np.asarray(static_feat, f32)
    Wk, bk = np.asarray(Wk, f32), np.asarray(bk, f32)
    Wv, bv = np.asarray(Wv, f32), np.asarray(bv, f32)
    Wout, bout = np.asarray(Wout, f32), np.asarray(bout, f32)

    Ek = np.einsum('oc,ncm->nom', np.asarray(WEk, f32), static_feat,
                   optimize=True) + np.asarray(bEk, f32)[None, :, None]
    Ev = np.einsum('oc,ncm->nom', np.asarray(WEv, f32), static_feat,
                   optimize=True) + np.asarray(bEv, f32)[None, :, None]
    Ekn = Ek / np.maximum(np.linalg.norm(Ek, axis=1, keepdims=True), EPS)
    Ekn_mat = Ekn.transpose(1, 0, 2).reshape(CH, J)          # [CH, 672]
    A_mat = Ev.transpose(0, 2, 1).reshape(J, CH)             # [672, CH]
    evwb = np.einsum('nom,o->nm', Ev, np.asarray(Ww, f32)[0]).reshape(J)

    # norm sketch + folded dots
    S = np.random.RandomState(0).randn(RSK, CH).astype(f32) / np.sqrt(RSK)
    W_sk = S @ Wk                                            # [128, CIN]
    b_sk = S @ bk
    D = Wk.T @ Ekn_mat                                       # [CIN, J]
    bkE = bk @ Ekn_mat                                       # [J]

    def cinlayout(w, width):    # [CIN, width] -> [P, KB*width]
        return np.ascontiguousarray(
            w.reshape(KB, P, width).transpose(1, 0, 2).reshape(P, KB * width))

    wsk8_h = cinlayout((W_sk.T * SD).astype(f8), RSK)
    d8_h = cinlayout((D * SD).astype(f8), J)

    def wlayout(w):     # [CIN, OCH] f8 -> dram [OCH/P, P, KB*P]
        och = w.shape[1]
        return np.ascontiguousarray(
            w.reshape(KB, P, och // P, P).transpose(2, 1, 0, 3)
            .reshape(och // P, P, KB * P))

    wv_s = Wv.T * SW
    wv8_f = wv_s.astype(f8)
    wv8lo_f = (wv_s - wv8_f.astype(f32)).astype(f8)
    wv8_h = wlayout(wv8_f)
    wv8lo_h = wlayout(wv8lo_f)

    a_pad = np.zeros((6 * P, CH), f32)
    a_pad[:J] = A_mat * SA
    a8_h = np.ascontiguousarray(
        a_pad.astype(f8).reshape(6, P, CH).transpose(1, 0, 2).reshape(P, 6 * CH))
    evwb_h = np.ascontiguousarray(
        np.broadcast_to(evwb.astype(bf16)[None, :], (P, J)))
    bke_h = (bkE * SD).astype(bf16).reshape(1, J)
    wout_sc = Wout.T.reshape(KB, P, NCLS).copy()
    for o in range(1, CHB, 2):       # hv blocks evicted via DVE carry x32
        wout_sc[o] /= SW
    wout_h = np.ascontiguousarray(
        wout_sc.transpose(1, 0, 2).reshape(P, KB * NCLS).astype(bf16))
    bsk_h = np.zeros((P, 1), f32)
    bsk_h[:RSK, 0] = b_sk * SD
    bv_h = np.ascontiguousarray(
        np.concatenate([bv.reshape(CHB, P).T, (bv * SW).reshape(CHB, P).T], 1))
    bout_h = bout.reshape(NCLS, 1)
    ident_h = np.eye(P, dtype=bf16)

    xT = np.ascontiguousarray(x[:, -1, :].T)                 # [CIN, B]
    x8_f = xT.astype(f8)
    x8lo_f = (xT - x8_f.astype(f32)).astype(f8)

    shared = dict(wsk8=wsk8_h, d8=d8_h, wv8=wv8_h, wv8lo=wv8lo_h, a8=a8_h,
                  evwb=evwb_h, bke=bke_h, wout=wout_h, bsk=bsk_h, bv=bv_h,
                  bout=bout_h, ident=ident_h)
    in_maps = []
    for c in range(NCORES):
        sl = slice(c * BL, (c + 1) * BL)

        def xlayout(xf):
            return np.ascontiguousarray(
                xf[:, sl].reshape(KB, P, BL).transpose(1, 0, 2)
                .reshape(P, KB * BL))
        in_maps.append(dict(x8=xlayout(x8_f), x8lo=xlayout(x8lo_f), **shared))
    return in_maps


_NC_CACHE = {}


def get_nc(debug=False):
    if debug not in _NC_CACHE:
        _NC_CACHE[debug] = build_nc(debug=debug)
    return _NC_CACHE[debug]


def kernel(**inputs) -> np.ndarray:
    nc = get_nc()
    in_maps = host_prep(**inputs)
    res = run_bass_kernel_spmd(nc, in_maps, list(range(NCORES)))
    out = np.empty((B, NCLS, 1), dtype=np.float32)
    for c in range(NCORES):
        out[c * BL:(c + 1) * BL, :, 0] = res.results[c]["out"].T
    return out


# revision 41
# speedup vs baseline: 1.0225x; 1.0027x over previous
"""Trainium2 Bass kernel for nn_Colar_static (retrieval_knn).

Sharding: data-parallel over batch B=2048 across 8 NeuronCores (256 rows each).
Weights/exemplars replicated, precomputed + quantized on host.

Design (vs the bf16 baseline at 53.3us):
  * Every large matmul is fp8e4m3 with the DoubleRow perf mode (K=256 per
    instruction, 0.5 cycles/row) -> 4x bf16 MAC rate and 1-byte weights
    (the kernel is DMA-bound: all DMA serializes at ~332 GB/s).
  * dots = x @ (Wk^T Ekn) directly: Wk is folded into the exemplars on the
    host, so the 2MB Wk and 0.7MB Ekn never ship; only D8 [CIN,672] (1.4MB).
  * ||k|| (softmax temperature only) via a random sketch: ||S k|| with
    S [128,1024] Gaussian, W_sk = S Wk [128, CIN] fp8 (0.25MB). The 5% norm
    error is invisible downstream (validated: rel err 3.21e-3, same as the
    exact-norm pipeline, because cos logits are tiny and softmax-smoothed).
  * v  = x8@Wv8hi + x8lo@Wv8hi + x8@Wv8lo   3-pass residual-compensated fp8
    (v dominates the output; plain fp8 fails at 3.3e-2).
  * fE = A8^T @ ut8 (fp8 DR);  out = Wout^T @ [hv;hfe] in bf16 (tiny).

Scales (all folded, no extra device work): D,W_sk x64; Wv x32; A x16; u x256.
The sketch scale cancels: rinv = rsqrt(sum((64 S k)^2)) = 1/(64||Sk||) and
dots are x64, so exp(dots*rinv) = exp(cos).

Rel err vs fp32 reference ~3.2e-3 (numpy-sim validated; gate is 2e-2).
"""

import numpy as np
import ml_dtypes

import concourse.bass as bass
import concourse.bacc as bacc
import concourse.mybir as mybir
import concourse.tile as tile
from concourse.bass_utils import run_bass_kernel_spmd

AF = mybir.ActivationFunctionType
BF = mybir.dt.bfloat16
F8 = mybir.dt.float8e4
F32 = mybir.dt.float32
DR = mybir.MatmulPerfMode.DoubleRow
bf16 = ml_dtypes.bfloat16
f8 = ml_dtypes.float8_e4m3

# Problem constants (hardcoded; kernel.py must be self-contained)
B, T, CIN, CH, M, NCLS = 2048, 8, 2048, 1024, 32, 21
NCORES = 8
BL = B // NCORES          # 256 batch rows per core
J = NCLS * M              # 672
P = 128
KB = CIN // P             # 16 contraction blocks over CIN
KP = KB // 2              # 8 DoubleRow pairs over CIN
CHB = CH // P             # 8 blocks over CH
NB = BL // P              # 2 batch chunks of 128
RSK = 64                  # norm-sketch rank
SD, SW, SA, SU = 64.0, 32.0, 16.0, 256.0
JC = [(0, 256), (256, 512), (512, J)]   # dots psum chunks (bank-safe)


def build_nc(debug=False):
    nc = bacc.Bacc("TRN2", target_bir_lowering=False, debug=debug,
                   num_devices=NCORES)

    x8_e = nc.dram_tensor("x8", [P, KB * BL], F8, kind="ExternalInput")
    x8lo_e = nc.dram_tensor("x8lo", [P, KB * BL], F8, kind="ExternalInput")
    wsk8_e = nc.dram_tensor("wsk8", [P, KB * RSK], F8, kind="ExternalInput")
    d8_e = nc.dram_tensor("d8", [P, KB * J], F8, kind="ExternalInput")
    wv8_e = nc.dram_tensor("wv8", [CHB, P, KB * P], F8, kind="ExternalInput")
    wv8lo_e = nc.dram_tensor("wv8lo", [CHB, P, KB * P], F8, kind="ExternalInput")
    a8_e = nc.dram_tensor("a8", [P, 6 * CH], F8, kind="ExternalInput")
    evwb_e = nc.dram_tensor("evwb", [P, J], BF, kind="ExternalInput")
    bke_e = nc.dram_tensor("bke", [1, J], BF, kind="ExternalInput")
    wout_e = nc.dram_tensor("wout", [P, KB * NCLS], BF, kind="ExternalInput")
    bsk_e = nc.dram_tensor("bsk", [P, 1], F32, kind="ExternalInput")
    bv_e = nc.dram_tensor("bv", [P, 2 * CHB], F32, kind="ExternalInput")
    bout_e = nc.dram_tensor("bout", [NCLS, 1], F32, kind="ExternalInput")
    ident_e = nc.dram_tensor("ident", [P, P], BF, kind="ExternalInput")
    out_e = nc.dram_tensor("out", [NCLS, BL], F32, kind="ExternalOutput")

    with tile.TileContext(nc) as tc:
        from contextlib import ExitStack
        with ExitStack() as ctx:
            pers = ctx.enter_context(tc.tile_pool(name="pers", bufs=1))
            pmisc = ctx.enter_context(tc.tile_pool(name="pmisc", bufs=1, space="PSUM"))
            pkv = ctx.enter_context(tc.tile_pool(name="pkv", bufs=2, space="PSUM"))
            pdot = ctx.enter_context(tc.tile_pool(name="pdot", bufs=1, space="PSUM"))
            ptr = ctx.enter_context(tc.tile_pool(name="ptr", bufs=1, space="PSUM"))
            pfe = ctx.enter_context(tc.tile_pool(name="pfe", bufs=1, space="PSUM"))

            # ---- SBUF tiles ----
            x8_s = pers.tile([P, KB, BL], F8, tag="x8")
            x8lo_s = pers.tile([P, KB, BL], F8, tag="x8lo")
            wsk8_s = pers.tile([P, KB, RSK], F8, tag="wsk8")
            d8_s = pers.tile([P, KB, J], F8, tag="d8")
            wv8_s = pers.tile([P, CHB, KB, P], F8, tag="wv8")
            wv8lo_s = pers.tile([P, CHB, KB, P], F8, tag="wv8lo")
            a8_s = pers.tile([P, 6, CH], F8, tag="a8")
            evwb_s = pers.tile([P, J], BF, tag="evwb")
            bke_s = pers.tile([1, J], BF, tag="bke")
            wout_s = pers.tile([P, KB, NCLS], BF, tag="wout")
            bsk_s = pers.tile([P, 1], F32, tag="bsk")
            bv_s = pers.tile([P, 2 * CHB], F32, tag="bv")
            bout_s = pers.tile([NCLS, 1], F32, tag="bout")
            ident_s = pers.tile([P, P], BF, tag="ident")
            ones_s = pers.tile([P, 1], BF, tag="ones")
            ones1_s = pers.tile([1, P], BF, tag="ones1")
            scratch_s = pers.tile([1, 1], F32, tag="scratch")
            sk_s = pers.tile([P, BL], BF, tag="sk")
            sksq_s = pers.tile([P, BL], BF, tag="sksq")
            hv_s = pers.tile([P, CHB, BL], BF, tag="hv")
            hfe_s = pers.tile([P, CHB, BL], BF, tag="hfe")
            e_s = pers.tile([P, NB, J], BF, tag="e")
            tmp_s = pers.tile([P, J], BF, tag="tmp")
            u_s = pers.tile([P, NB, J], BF, tag="u")
            ut_s = pers.tile([P, 6, BL], F8, tag="ut")
            rinv_s = pers.tile([P, NB], F32, tag="rinv")
            rs1_s = pers.tile([P, NB], F32, tag="rs1")
            rs2_s = pers.tile([P, NB], F32, tag="rs2")
            magic_s = pers.tile([P, 1], mybir.dt.int32, tag="magic")
            s_s = pers.tile([P, NB * NCLS], BF, tag="s")
            num_s = pers.tile([P, NB * NCLS], BF, tag="num")
            sinv_s = pers.tile([P, NB * NCLS], F32, tag="sinv")
            t_s = pers.tile([P, NB * NCLS], F32, tag="t")
            g_s = pers.tile([P, NB * NCLS], F32, tag="g")
            gg_s = pers.tile([P, NB], F32, tag="gg")
            ginv_s = pers.tile([P, NB], F32, tag="ginv")
            c1_s = pers.tile([P, NB * NCLS], F32, tag="c1")
            c_s = pers.tile([P, NB * NCLS], F32, tag="c")
            out_sb = pers.tile([NCLS, BL], F32, tag="outsb")

            # ---- setup: memsets + pin the Exp ACT table before any evict ----
            nc.vector.memset(ones_s[:], 1.0)
            nc.vector.memset(ones1_s[:], 1.0)
            nc.vector.memset(magic_s[:], 0x5f3759df)
            nc.vector.memset(ut_s[:], 0.0)        # zero jb-5 pad partitions
            nc.vector.memset(scratch_s[:], 1.0)
            nc.scalar.activation(scratch_s[:], scratch_s[:], AF.Exp)

            # ---- DMA schedule (sync queue; DMA device is the critical
            # resource at ~22.6us busy). wv pairs last: each gates only one
            # v pass. ----
            nc.sync.dma_start(x8_s[:], x8_e.ap())
            nc.sync.dma_start(wsk8_s[:], wsk8_e.ap())
            nc.sync.dma_start(bke_s[:], bke_e.ap())
            nc.sync.dma_start(evwb_s[:], evwb_e.ap())
            nc.sync.dma_start(d8_s[:], d8_e.ap())
            nc.sync.dma_start(x8lo_s[:], x8lo_e.ap())
            nc.sync.dma_start(wout_s[:], wout_e.ap())
            for o in range(CHB):
                nc.sync.dma_start(wv8_s[:, o, :, :], wv8_e.ap()[o])
                nc.sync.dma_start(wv8lo_s[:, o, :, :], wv8lo_e.ap()[o])
                if o == 1:
                    nc.sync.dma_start(a8_s[:], a8_e.ap())
            nc.gpsimd.dma_start(bsk_s[:], bsk_e.ap())
            nc.gpsimd.dma_start(bv_s[:], bv_e.ap())
            nc.gpsimd.dma_start(bout_s[:], bout_e.ap())
            nc.gpsimd.dma_start(ident_s[:], ident_e.ap())

            # ---- phase 1: norm sketch: sk = 64*S*k, rinv = 1/||sk|| ----
            ps = pkv.tile([P, BL], F32, tag="pkv")
            for p in range(KP):
                nc.tensor.matmul(ps[0:RSK, :], wsk8_s[:, 2 * p:2 * p + 2, :],
                                 x8_s[:, 2 * p:2 * p + 2, :],
                                 start=(p == 0), stop=(p == KP - 1),
                                 perf_mode=DR)
            nc.scalar.activation(sk_s[0:RSK, :], ps[0:RSK, :], AF.Identity,
                                 bias=bsk_s[0:RSK, :])
            nc.vector.tensor_mul(sksq_s[0:RSK, :], sk_s[0:RSK, :],
                                 sk_s[0:RSK, :])
            ps2 = pmisc.tile([P, NB], F32, tag="misc")
            for bc in range(NB):
                nc.tensor.matmul(ps2[:, bc:bc + 1],
                                 sksq_s[0:RSK, bc * P:(bc + 1) * P],
                                 ones_s[0:RSK, :],
                                 start=True, stop=True)
                sq = rs1_s[:, bc:bc + 1]
                nc.vector.tensor_copy(sq, ps2[:, bc:bc + 1])
                y = rinv_s[:, bc:bc + 1]
                nc.vector.tensor_scalar(
                    y.bitcast(mybir.dt.int32), sq.bitcast(mybir.dt.int32),
                    1, None, op0=mybir.AluOpType.logical_shift_right)
                nc.vector.tensor_tensor(
                    out=y.bitcast(mybir.dt.int32), in0=magic_s[:],
                    in1=y.bitcast(mybir.dt.int32),
                    op=mybir.AluOpType.subtract)
                for _ in range(2):
                    t1 = rs2_s[:, bc:bc + 1]
                    nc.vector.tensor_mul(t1, y, y)
                    nc.vector.tensor_mul(t1, t1, sq)
                    nc.vector.tensor_scalar(t1, t1, -0.5, 1.5,
                                            op0=mybir.AluOpType.mult,
                                            op1=mybir.AluOpType.add)
                    nc.vector.tensor_mul(y, y, t1)

            # ---- phase 2: dots = x8 @ D8 (+bkE), chunk-major fp8 DR ----
            def dots(bc):
                psd = pdot.tile([P, J], F32, tag="pdot")
                for (c0, c1) in JC:
                    for p in range(KP):
                        nc.tensor.matmul(
                            psd[:, c0:c1],
                            x8_s[:, 2 * p:2 * p + 2, bc * P:bc * P + P],
                            d8_s[:, 2 * p:2 * p + 2, c0:c1],
                            start=(p == 0), stop=False, perf_mode=DR)
                    # += bkE (K=1 rank-1 broadcast matmul closes the group)
                    nc.tensor.matmul(psd[:, c0:c1], ones1_s[:],
                                     bke_s[:, c0:c1], start=False, stop=True)
                nc.scalar.activation(e_s[:, bc, 0:512], psd[:, 0:512], AF.Exp,
                                     scale=rinv_s[:, bc:bc + 1])
                nc.scalar.activation(e_s[:, bc, 512:J], psd[:, 512:J], AF.Exp,
                                     scale=rinv_s[:, bc:bc + 1])

            def softmax_chain(bc, eng):
                lp = nc.allow_low_precision(
                    reason="S/num feed softmax ratios; errors attenuate")
                lp.__enter__()
                # bc0 runs on DVE, bc1 on gpsimd: the two chains execute in
                # parallel so u1 lands ~2us earlier
                e_sl = e_s[:, bc, :]
                e3 = e_sl.rearrange("p (n m) -> p n m", m=M)
                ncls_sl = slice(bc * NCLS, (bc + 1) * NCLS)
                s2 = s_s[:, ncls_sl]
                eng.reduce_sum(s2, e3, axis=mybir.AxisListType.X)
                u_tmp = u_s[:, bc, :]
                eng.tensor_mul(u_tmp, e_sl, evwb_s[:])
                eng.reduce_sum(num_s[:, ncls_sl],
                               u_tmp.rearrange("p (n m) -> p n m", m=M),
                               axis=mybir.AxisListType.X)
                eng.reciprocal(sinv_s[:, ncls_sl], s2)
                eng.tensor_mul(t_s[:, ncls_sl], num_s[:, ncls_sl],
                               sinv_s[:, ncls_sl])
                nc.scalar.activation(g_s[:, ncls_sl], t_s[:, ncls_sl], AF.Exp)
                eng.reduce_sum(gg_s[:, bc:bc + 1], g_s[:, ncls_sl],
                               axis=mybir.AxisListType.X)
                eng.reciprocal(ginv_s[:, bc:bc + 1], gg_s[:, bc:bc + 1])
                # fold the u scale SU into ginv: c = g*sinv * (SU/G)
                eng.tensor_scalar(ginv_s[:, bc:bc + 1], ginv_s[:, bc:bc + 1],
                                  SU, None, op0=mybir.AluOpType.mult)
                eng.tensor_mul(c1_s[:, ncls_sl], g_s[:, ncls_sl],
                               sinv_s[:, ncls_sl])
                eng.tensor_scalar_mul(c_s[:, ncls_sl], c1_s[:, ncls_sl],
                                      ginv_s[:, bc:bc + 1])
                c_b = bass.AP(c_s.tensor, c_s[:, ncls_sl].offset,
                              c_s[:, ncls_sl].ap + [[0, M]])
                u3 = u_s[:, bc, :].rearrange("p (n m) -> p n m", m=M)
                eng.tensor_mul(u3, e3, c_b)
                lp.__exit__(None, None, None)

            # ---- out accumulator: block matmuls emitted as inputs land ----
            pso = pmisc.tile([NCLS, BL], F32, tag="misc")
            n_out_mm = [0]

            def out_mm(h_s, i):
                kb = i if h_s is hv_s else CHB + i
                nc.tensor.matmul(pso[:], wout_s[:, kb, :], h_s[:, i, :],
                                 start=(n_out_mm[0] == 0),
                                 stop=(n_out_mm[0] == KB - 1))
                n_out_mm[0] += 1

            def v_block(o):
                ps = pkv.tile([P, BL], F32, tag="pkv")
                n = 0
                for (wt, xt) in ((wv8_s, x8_s), (wv8_s, x8lo_s),
                                 (wv8lo_s, x8_s)):
                    for p in range(KP):
                        nc.tensor.matmul(ps[:], wt[:, o, 2 * p:2 * p + 2, :],
                                         xt[:, 2 * p:2 * p + 2, :],
                                         start=(n == 0), stop=(n == 3 * KP - 1),
                                         perf_mode=DR)
                        n += 1
                if o % 2 == 0:
                    nc.scalar.activation(hv_s[:, o, :], ps[:], AF.Relu,
                                         scale=1.0 / SW, bias=bv_s[:, o:o + 1])
                else:
                    # DVE path: max(psum + 32*bv, 0) = 32*hv; the 1/32 is
                    # folded into this block's wout column on the host
                    nc.vector.tensor_scalar(hv_s[:, o, :], ps[:],
                                            bv_s[:, CHB + o:CHB + o + 1], 0.0,
                                            op0=mybir.AluOpType.add,
                                            op1=mybir.AluOpType.max)

            def transpose_u(bc):
                def tgroup(grp, pool, ptag):
                    pst = pool.tile([P, 3 * P], BF, tag=ptag)
                    for t, jb in enumerate(grp):
                        w = P if jb < 5 else J - 5 * P
                        nc.tensor.transpose(
                            pst[:w, t * P:(t + 1) * P],
                            u_s[:, bc, jb * P:jb * P + w],
                            ident_s[:])
                    n = sum(1 for jb in grp if jb < 5)
                    base = ut_s[:, grp[0], bc * P:bc * P + P]
                    dst = bass.AP(ut_s.tensor, base.offset,
                                  [base.ap[0], [BL, n], base.ap[1]])
                    # ACT, not DVE: the DVE queue is still draining the
                    # softmax chain when these evicts become ready
                    nc.scalar.activation(
                        dst, pst[:, 0:n * P].rearrange("p (n q) -> p n q", q=P),
                        AF.Identity)
                    if n < len(grp):
                        jb = grp[n]
                        w = J - 5 * P
                        nc.scalar.activation(
                            ut_s[:w, jb, bc * P:bc * P + P],
                            pst[:w, n * P:(n + 1) * P], AF.Identity)
                # alternate psum pools so the four transpose groups pipeline
                tgroup((0, 1, 2), ptr, "ptr")
                tgroup((3, 4, 5), pfe, f"pfe{bc}")

            def fe_all():
                # rotate through three psum slots (pdot is idle after exps)
                for o in range(CHB):
                    if o % 3 < 2:
                        acc = pfe.tile([P, BL], F32, tag=f"pfe{o % 3}")
                    else:
                        acc = pdot.tile([P, BL], F32, tag="pdot")
                    for t in range(3):
                        nc.tensor.matmul(acc[:], a8_s[:, 2 * t:2 * t + 2,
                                                      o * P:(o + 1) * P],
                                         ut_s[:, 2 * t:2 * t + 2, :],
                                         start=(t == 0), stop=(t == 2),
                                         perf_mode=DR)
                    dst = hfe_s[:, o, :]
                    if o % 2 == 0:
                        nc.scalar.activation(dst, acc[:], AF.Relu,
                                             scale=1.0 / (SA * SU))
                    else:
                        nc.vector.tensor_scalar(dst, acc[:],
                                                1.0 / (SA * SU), 0.0,
                                                op0=mybir.AluOpType.mult,
                                                op1=mybir.AluOpType.max)

            # ---- main interleave: the whole dots/softmax/transpose/fE chain
            # runs before the wv stream thickens; v blocks then track DMA ----
            dots(0)
            softmax_chain(0, nc.vector)
            dots(1)
            softmax_chain(1, nc.vector)
            v_block(0)
            v_block(1)
            out_mm(hv_s, 0)
            transpose_u(0)
            transpose_u(1)
            fe_all()
            for i in range(CHB):
                out_mm(hfe_s, i)
            for o in range(2, CHB):
                v_block(o)
                out_mm(hv_s, o - 1)
            out_mm(hv_s, CHB - 1)

            # ---- +bout, DMA out ----
            nc.vector.tensor_scalar_add(out_sb[:], pso[:], bout_s[:, 0:1])
            nc.sync.dma_start(out_e.ap(), out_sb[:])

    nc.compile()
    return nc


def host_prep(x, static_feat, Wk, bk, Wv, bv, WEk, bEk, WEv, bEv, Ww, bw,
              Wout, bout):
    """Host-side fp32 precompute, fp8/bf16 quantization, per-core input maps."""
    EPS = 1e-8
    f32 = np.float32
    x = np.asarray(x, f32)
    static_feat = np.asarray(static_feat, f32)
    Wk, bk = np.asarray(Wk, f32), np.asarray(bk, f32)
    Wv, bv = np.asarray(Wv, f32), np.asarray(bv, f32)
    Wout, bout = np.asarray(Wout, f32), np.asarray(bout, f32)

    Ek = np.einsum('oc,ncm->nom', np.asarray(WEk, f32), static_feat,
                   optimize=True) + np.asarray(bEk, f32)[None, :, None]
    Ev = np.einsum('oc,ncm->nom', np.asarray(WEv, f32), static_feat,
                   optimize=True) + np.asarray(bEv, f32)[None, :, None]
    Ekn = Ek / np.maximum(np.linalg.norm(Ek, axis=1, keepdims=True), EPS)
    Ekn_mat = Ekn.transpose(1, 0, 2).reshape(CH, J)          # [CH, 672]
    A_mat = Ev.transpose(0, 2, 1).reshape(J, CH)             # [672, CH]
    evwb = np.einsum('nom,o->nm', Ev, np.asarray(Ww, f32)[0]).reshape(J)

    # norm sketch + folded dots
    S = np.random.RandomState(0).randn(RSK, CH).astype(f32) / np.sqrt(RSK)
    W_sk = S @ Wk                                            # [128, CIN]
    b_sk = S @ bk
    D = Wk.T @ Ekn_mat                                       # [CIN, J]
    bkE = bk @ Ekn_mat                                       # [J]

    def cinlayout(w, width):    # [CIN, width] -> [P, KB*width]
        return np.ascontiguousarray(
            w.reshape(KB, P, width).transpose(1, 0, 2).reshape(P, KB * width))

    wsk8_h = cinlayout((W_sk.T * SD).astype(f8), RSK)
    d8_h = cinlayout((D * SD).astype(f8), J)

    def wlayout(w):     # [CIN, OCH] f8 -> dram [OCH/P, P, KB*P]
        och = w.shape[1]
        return np.ascontiguousarray(
            w.reshape(KB, P, och // P, P).transpose(2, 1, 0, 3)
            .reshape(och // P, P, KB * P))

    wv_s = Wv.T * SW
    wv8_f = wv_s.astype(f8)
    wv8lo_f = (wv_s - wv8_f.astype(f32)).astype(f8)
    wv8_h = wlayout(wv8_f)
    wv8lo_h = wlayout(wv8lo_f)

    a_pad = np.zeros((6 * P, CH), f32)
    a_pad[:J] = A_mat * SA
    a8_h = np.ascontiguousarray(
        a_pad.astype(f8).reshape(6, P, CH).transpose(1, 0, 2).reshape(P, 6 * CH))
    evwb_h = np.ascontiguousarray(
        np.broadcast_to(evwb.astype(bf16)[None, :], (P, J)))
    bke_h = (bkE * SD).astype(bf16).reshape(1, J)
    wout_sc = Wout.T.reshape(KB, P, NCLS).copy()
    for o in range(1, CHB, 2):       # hv blocks evicted via DVE carry x32
        wout_sc[o] /= SW
    wout_h = np.ascontiguousarray(
        wout_sc.transpose(1, 0, 2).reshape(P, KB * NCLS).astype(bf16))
    bsk_h = np.zeros((P, 1), f32)
    bsk_h[:RSK, 0] = b_sk * SD
    bv_h = np.ascontiguousarray(
        np.concatenate([bv.reshape(CHB, P).T, (bv * SW).reshape(CHB, P).T], 1))
    bout_h = bout.reshape(NCLS, 1)
    ident_h = np.eye(P, dtype=bf16)

    xT = np.ascontiguousarray(x[:, -1, :].T)                 # [CIN, B]
    x8_f = xT.astype(f8)
    x8lo_f = (xT - x8_f.astype(f32)).astype(f8)

    shared = dict(wsk8=wsk8_h, d8=d8_h, wv8=wv8_h, wv8lo=wv8lo_h, a8=a8_h,
                  evwb=evwb_h, bke=bke_h, wout=wout_h, bsk=bsk_h, bv=bv_h,
                  bout=bout_h, ident=ident_h)
    in_maps = []
    for c in range(NCORES):
        sl = slice(c * BL, (c + 1) * BL)

        def xlayout(xf):
            return np.ascontiguousarray(
                xf[:, sl].reshape(KB, P, BL).transpose(1, 0, 2)
                .reshape(P, KB * BL))
        in_maps.append(dict(x8=xlayout(x8_f), x8lo=xlayout(x8lo_f), **shared))
    return in_maps


_NC_CACHE = {}


def get_nc(debug=False):
    if debug not in _NC_CACHE:
        _NC_CACHE[debug] = build_nc(debug=debug)
    return _NC_CACHE[debug]


def kernel(**inputs) -> np.ndarray:
    nc = get_nc()
    in_maps = host_prep(**inputs)
    res = run_bass_kernel_spmd(nc, in_maps, list(range(NCORES)))
    out = np.empty((B, NCLS, 1), dtype=np.float32)
    for c in range(NCORES):
        out[c * BL:(c + 1) * BL, :, 0] = res.results[c]["out"].T
    return out


# revision 42
# speedup vs baseline: 1.0245x; 1.0019x over previous
"""Trainium2 Bass kernel for nn_Colar_static (retrieval_knn).

Sharding: data-parallel over batch B=2048 across 8 NeuronCores (256 rows each).
Weights/exemplars replicated, precomputed + quantized on host.

Design (vs the bf16 baseline at 53.3us):
  * Every large matmul is fp8e4m3 with the DoubleRow perf mode (K=256 per
    instruction, 0.5 cycles/row) -> 4x bf16 MAC rate and 1-byte weights
    (the kernel is DMA-bound: all DMA serializes at ~332 GB/s).
  * dots = x @ (Wk^T Ekn) directly: Wk is folded into the exemplars on the
    host, so the 2MB Wk and 0.7MB Ekn never ship; only D8 [CIN,672] (1.4MB).
  * ||k|| (softmax temperature only) via a random sketch: ||S k|| with
    S [128,1024] Gaussian, W_sk = S Wk [128, CIN] fp8 (0.25MB). The 5% norm
    error is invisible downstream (validated: rel err 3.21e-3, same as the
    exact-norm pipeline, because cos logits are tiny and softmax-smoothed).
  * v  = x8@Wv8hi + x8lo@Wv8hi + x8@Wv8lo   3-pass residual-compensated fp8
    (v dominates the output; plain fp8 fails at 3.3e-2).
  * fE = A8^T @ ut8 (fp8 DR);  out = Wout^T @ [hv;hfe] in bf16 (tiny).

Scales (all folded, no extra device work): D,W_sk x64; Wv x32; A x16; u x256.
The sketch scale cancels: rinv = rsqrt(sum((64 S k)^2)) = 1/(64||Sk||) and
dots are x64, so exp(dots*rinv) = exp(cos).

Rel err vs fp32 reference ~3.2e-3 (numpy-sim validated; gate is 2e-2).
"""

import numpy as np
import ml_dtypes

import concourse.bass as bass
import concourse.bacc as bacc
import concourse.mybir as mybir
import concourse.tile as tile
from concourse.bass_utils import run_bass_kernel_spmd

AF = mybir.ActivationFunctionType
BF = mybir.dt.bfloat16
F8 = mybir.dt.float8e4
F32 = mybir.dt.float32
DR = mybir.MatmulPerfMode.DoubleRow
bf16 = ml_dtypes.bfloat16
f8 = ml_dtypes.float8_e4m3

# Problem constants (hardcoded; kernel.py must be self-contained)
B, T, CIN, CH, M, NCLS = 2048, 8, 2048, 1024, 32, 21
NCORES = 8
BL = B // NCORES          # 256 batch rows per core
J = NCLS * M              # 672
P = 128
KB = CIN // P             # 16 contraction blocks over CIN
KP = KB // 2              # 8 DoubleRow pairs over CIN
CHB = CH // P             # 8 blocks over CH
NB = BL // P              # 2 batch chunks of 128
RSK = 64                  # norm-sketch rank
SD, SW, SA, SU = 64.0, 32.0, 16.0, 256.0
JC = [(0, 256), (256, 512), (512, J)]   # dots psum chunks (bank-safe)


def build_nc(debug=False):
    nc = bacc.Bacc("TRN2", target_bir_lowering=False, debug=debug,
                   num_devices=NCORES)

    x8_e = nc.dram_tensor("x8", [P, KB * BL], F8, kind="ExternalInput")
    x8lo_e = nc.dram_tensor("x8lo", [P, KB * BL], F8, kind="ExternalInput")
    wsk8_e = nc.dram_tensor("wsk8", [P, KB * RSK], F8, kind="ExternalInput")
    d8_e = nc.dram_tensor("d8", [P, KB * J], F8, kind="ExternalInput")
    wv8_e = nc.dram_tensor("wv8", [CHB, P, KB * P], F8, kind="ExternalInput")
    wv8lo_e = nc.dram_tensor("wv8lo", [CHB, P, KB * P], F8, kind="ExternalInput")
    a8_e = nc.dram_tensor("a8", [P, 6 * CH], F8, kind="ExternalInput")
    evwb_e = nc.dram_tensor("evwb", [P, J], BF, kind="ExternalInput")
    bke_e = nc.dram_tensor("bke", [1, J], BF, kind="ExternalInput")
    wout_e = nc.dram_tensor("wout", [P, KB * NCLS], BF, kind="ExternalInput")
    bsk_e = nc.dram_tensor("bsk", [P, 1], F32, kind="ExternalInput")
    bv_e = nc.dram_tensor("bv", [P, 2 * CHB], F32, kind="ExternalInput")
    bout_e = nc.dram_tensor("bout", [NCLS, 1], F32, kind="ExternalInput")
    ident_e = nc.dram_tensor("ident", [P, P], BF, kind="ExternalInput")
    out_e = nc.dram_tensor("out", [NCLS, BL], F32, kind="ExternalOutput")

    with tile.TileContext(nc) as tc:
        from contextlib import ExitStack
        with ExitStack() as ctx:
            pers = ctx.enter_context(tc.tile_pool(name="pers", bufs=1))
            pmisc = ctx.enter_context(tc.tile_pool(name="pmisc", bufs=1, space="PSUM"))
            pkv = ctx.enter_context(tc.tile_pool(name="pkv", bufs=2, space="PSUM"))
            pdot = ctx.enter_context(tc.tile_pool(name="pdot", bufs=1, space="PSUM"))
            ptr = ctx.enter_context(tc.tile_pool(name="ptr", bufs=1, space="PSUM"))
            pfe = ctx.enter_context(tc.tile_pool(name="pfe", bufs=1, space="PSUM"))

            # ---- SBUF tiles ----
            x8_s = pers.tile([P, KB, BL], F8, tag="x8")
            x8lo_s = pers.tile([P, KB, BL], F8, tag="x8lo")
            wsk8_s = pers.tile([P, KB, RSK], F8, tag="wsk8")
            d8_s = pers.tile([P, KB, J], F8, tag="d8")
            wv8_s = pers.tile([P, CHB, KB, P], F8, tag="wv8")
            wv8lo_s = pers.tile([P, CHB, KB, P], F8, tag="wv8lo")
            a8_s = pers.tile([P, 6, CH], F8, tag="a8")
            evwb_s = pers.tile([P, J], BF, tag="evwb")
            bke_s = pers.tile([1, J], BF, tag="bke")
            wout_s = pers.tile([P, KB, NCLS], BF, tag="wout")
            bsk_s = pers.tile([P, 1], F32, tag="bsk")
            bv_s = pers.tile([P, 2 * CHB], F32, tag="bv")
            bout_s = pers.tile([NCLS, 1], F32, tag="bout")
            ident_s = pers.tile([P, P], BF, tag="ident")
            ones_s = pers.tile([P, 1], BF, tag="ones")
            ones1_s = pers.tile([1, P], BF, tag="ones1")
            scratch_s = pers.tile([1, 1], F32, tag="scratch")
            sk_s = pers.tile([P, BL], BF, tag="sk")
            sksq_s = pers.tile([P, BL], BF, tag="sksq")
            hv_s = pers.tile([P, CHB, BL], BF, tag="hv")
            hfe_s = pers.tile([P, CHB, BL], BF, tag="hfe")
            e_s = pers.tile([P, NB, J], BF, tag="e")
            tmp_s = pers.tile([P, J], BF, tag="tmp")
            u_s = pers.tile([P, NB, J], BF, tag="u")
            ut_s = pers.tile([P, 6, BL], F8, tag="ut")
            rinv_s = pers.tile([P, NB], F32, tag="rinv")
            rs1_s = pers.tile([P, NB], F32, tag="rs1")
            rs2_s = pers.tile([P, NB], F32, tag="rs2")
            magic_s = pers.tile([P, 1], mybir.dt.int32, tag="magic")
            s_s = pers.tile([P, NB * NCLS], BF, tag="s")
            num_s = pers.tile([P, NB * NCLS], BF, tag="num")
            sinv_s = pers.tile([P, NB * NCLS], F32, tag="sinv")
            t_s = pers.tile([P, NB * NCLS], F32, tag="t")
            g_s = pers.tile([P, NB * NCLS], F32, tag="g")
            gg_s = pers.tile([P, NB], F32, tag="gg")
            ginv_s = pers.tile([P, NB], F32, tag="ginv")
            c1_s = pers.tile([P, NB * NCLS], F32, tag="c1")
            c_s = pers.tile([P, NB * NCLS], F32, tag="c")
            out_sb = pers.tile([NCLS, BL], F32, tag="outsb")

            # ---- setup: memsets + pin the Exp ACT table before any evict ----
            nc.vector.memset(ones_s[:], 1.0)
            nc.vector.memset(ones1_s[:], 1.0)
            nc.vector.memset(magic_s[:], 0x5f3759df)
            nc.vector.memset(ut_s[:], 0.0)        # zero jb-5 pad partitions
            nc.vector.memset(scratch_s[:], 1.0)
            nc.scalar.activation(scratch_s[:], scratch_s[:], AF.Exp)

            # ---- DMA schedule (sync queue; DMA device is the critical
            # resource at ~22.6us busy). wv pairs last: each gates only one
            # v pass. ----
            nc.sync.dma_start(x8_s[:], x8_e.ap())
            nc.sync.dma_start(wsk8_s[:], wsk8_e.ap())
            nc.sync.dma_start(bke_s[:], bke_e.ap())
            nc.sync.dma_start(evwb_s[:], evwb_e.ap())
            nc.sync.dma_start(d8_s[:], d8_e.ap())
            nc.sync.dma_start(x8lo_s[:], x8lo_e.ap())
            nc.sync.dma_start(wout_s[:], wout_e.ap())
            for o in range(CHB):
                nc.sync.dma_start(wv8_s[:, o, :, :], wv8_e.ap()[o])
                nc.sync.dma_start(wv8lo_s[:, o, :, :], wv8lo_e.ap()[o])
                if o == 1:
                    nc.sync.dma_start(a8_s[:], a8_e.ap())
            nc.gpsimd.dma_start(bsk_s[:], bsk_e.ap())
            nc.gpsimd.dma_start(bv_s[:], bv_e.ap())
            nc.gpsimd.dma_start(bout_s[:], bout_e.ap())
            nc.gpsimd.dma_start(ident_s[:], ident_e.ap())

            # ---- phase 1: norm sketch: sk = 64*S*k, rinv = 1/||sk|| ----
            ps = pkv.tile([P, BL], F32, tag="pkv")
            for p in range(KP):
                nc.tensor.matmul(ps[0:RSK, :], wsk8_s[:, 2 * p:2 * p + 2, :],
                                 x8_s[:, 2 * p:2 * p + 2, :],
                                 start=(p == 0), stop=(p == KP - 1),
                                 perf_mode=DR)
            nc.scalar.activation(sk_s[0:RSK, :], ps[0:RSK, :], AF.Identity,
                                 bias=bsk_s[0:RSK, :])
            nc.vector.tensor_mul(sksq_s[0:RSK, :], sk_s[0:RSK, :],
                                 sk_s[0:RSK, :])
            ps2 = pmisc.tile([P, NB], F32, tag="misc")
            for bc in range(NB):
                nc.tensor.matmul(ps2[:, bc:bc + 1],
                                 sksq_s[0:RSK, bc * P:(bc + 1) * P],
                                 ones_s[0:RSK, :],
                                 start=True, stop=True)
                sq = rs1_s[:, bc:bc + 1]
                nc.vector.tensor_copy(sq, ps2[:, bc:bc + 1])
                y = rinv_s[:, bc:bc + 1]
                nc.vector.tensor_scalar(
                    y.bitcast(mybir.dt.int32), sq.bitcast(mybir.dt.int32),
                    1, None, op0=mybir.AluOpType.logical_shift_right)
                nc.vector.tensor_tensor(
                    out=y.bitcast(mybir.dt.int32), in0=magic_s[:],
                    in1=y.bitcast(mybir.dt.int32),
                    op=mybir.AluOpType.subtract)
                for _ in range(2):
                    t1 = rs2_s[:, bc:bc + 1]
                    nc.vector.tensor_mul(t1, y, y)
                    nc.vector.tensor_mul(t1, t1, sq)
                    nc.vector.tensor_scalar(t1, t1, -0.5, 1.5,
                                            op0=mybir.AluOpType.mult,
                                            op1=mybir.AluOpType.add)
                    nc.vector.tensor_mul(y, y, t1)

            # ---- phase 2: dots = x8 @ D8 (+bkE), chunk-major fp8 DR ----
            def dots(bc):
                psd = pdot.tile([P, J], F32, tag="pdot")
                for (c0, c1) in JC:
                    for p in range(KP):
                        nc.tensor.matmul(
                            psd[:, c0:c1],
                            x8_s[:, 2 * p:2 * p + 2, bc * P:bc * P + P],
                            d8_s[:, 2 * p:2 * p + 2, c0:c1],
                            start=(p == 0), stop=False, perf_mode=DR)
                    # += bkE (K=1 rank-1 broadcast matmul closes the group)
                    nc.tensor.matmul(psd[:, c0:c1], ones1_s[:],
                                     bke_s[:, c0:c1], start=False, stop=True)
                nc.scalar.activation(e_s[:, bc, 0:512], psd[:, 0:512], AF.Exp,
                                     scale=rinv_s[:, bc:bc + 1])
                nc.scalar.activation(e_s[:, bc, 512:J], psd[:, 512:J], AF.Exp,
                                     scale=rinv_s[:, bc:bc + 1])

            def softmax_chain(bc, eng):
                lp = nc.allow_low_precision(
                    reason="S/num feed softmax ratios; errors attenuate")
                lp.__enter__()
                # bc0 runs on DVE, bc1 on gpsimd: the two chains execute in
                # parallel so u1 lands ~2us earlier
                e_sl = e_s[:, bc, :]
                e3 = e_sl.rearrange("p (n m) -> p n m", m=M)
                ncls_sl = slice(bc * NCLS, (bc + 1) * NCLS)
                s2 = s_s[:, ncls_sl]
                eng.reduce_sum(s2, e3, axis=mybir.AxisListType.X)
                u_tmp = u_s[:, bc, :]
                eng.tensor_mul(u_tmp, e_sl, evwb_s[:])
                eng.reduce_sum(num_s[:, ncls_sl],
                               u_tmp.rearrange("p (n m) -> p n m", m=M),
                               axis=mybir.AxisListType.X)
                eng.reciprocal(sinv_s[:, ncls_sl], s2)
                eng.tensor_mul(t_s[:, ncls_sl], num_s[:, ncls_sl],
                               sinv_s[:, ncls_sl])
                nc.scalar.activation(g_s[:, ncls_sl], t_s[:, ncls_sl], AF.Exp)
                eng.reduce_sum(gg_s[:, bc:bc + 1], g_s[:, ncls_sl],
                               axis=mybir.AxisListType.X)
                eng.reciprocal(ginv_s[:, bc:bc + 1], gg_s[:, bc:bc + 1])
                # fold the u scale SU into ginv: c = g*sinv * (SU/G)
                eng.tensor_scalar(ginv_s[:, bc:bc + 1], ginv_s[:, bc:bc + 1],
                                  SU, None, op0=mybir.AluOpType.mult)
                eng.tensor_mul(c1_s[:, ncls_sl], g_s[:, ncls_sl],
                               sinv_s[:, ncls_sl])
                eng.tensor_scalar_mul(c_s[:, ncls_sl], c1_s[:, ncls_sl],
                                      ginv_s[:, bc:bc + 1])
                c_b = bass.AP(c_s.tensor, c_s[:, ncls_sl].offset,
                              c_s[:, ncls_sl].ap + [[0, M]])
                u3 = u_s[:, bc, :].rearrange("p (n m) -> p n m", m=M)
                eng.tensor_mul(u3, e3, c_b)
                lp.__exit__(None, None, None)

            # ---- out accumulator: block matmuls emitted as inputs land ----
            pso = pmisc.tile([NCLS, BL], F32, tag="misc")
            n_out_mm = [0]

            def out_mm(h_s, i):
                kb = i if h_s is hv_s else CHB + i
                nc.tensor.matmul(pso[:], wout_s[:, kb, :], h_s[:, i, :],
                                 start=(n_out_mm[0] == 0),
                                 stop=(n_out_mm[0] == KB - 1))
                n_out_mm[0] += 1

            def v_block(o):
                ps = pkv.tile([P, BL], F32, tag="pkv")
                n = 0
                for (wt, xt) in ((wv8_s, x8_s), (wv8_s, x8lo_s),
                                 (wv8lo_s, x8_s)):
                    for p in range(KP):
                        nc.tensor.matmul(ps[:], wt[:, o, 2 * p:2 * p + 2, :],
                                         xt[:, 2 * p:2 * p + 2, :],
                                         start=(n == 0), stop=(n == 3 * KP - 1),
                                         perf_mode=DR)
                        n += 1
                if o % 2 == 0:
                    nc.scalar.activation(hv_s[:, o, :], ps[:], AF.Relu,
                                         scale=1.0 / SW, bias=bv_s[:, o:o + 1])
                else:
                    # DVE path: max(psum + 32*bv, 0) = 32*hv; the 1/32 is
                    # folded into this block's wout column on the host
                    nc.vector.tensor_scalar(hv_s[:, o, :], ps[:],
                                            bv_s[:, CHB + o:CHB + o + 1], 0.0,
                                            op0=mybir.AluOpType.add,
                                            op1=mybir.AluOpType.max)

            def transpose_u(bc):
                def tgroup(grp, pool, ptag):
                    pst = pool.tile([P, 3 * P], BF, tag=ptag)
                    for t, jb in enumerate(grp):
                        w = P if jb < 5 else J - 5 * P
                        nc.tensor.transpose(
                            pst[:w, t * P:(t + 1) * P],
                            u_s[:, bc, jb * P:jb * P + w],
                            ident_s[:])
                    n = sum(1 for jb in grp if jb < 5)
                    base = ut_s[:, grp[0], bc * P:bc * P + P]
                    dst = bass.AP(ut_s.tensor, base.offset,
                                  [base.ap[0], [BL, n], base.ap[1]])
                    # ACT, not DVE: the DVE queue is still draining the
                    # softmax chain when these evicts become ready
                    nc.scalar.activation(
                        dst, pst[:, 0:n * P].rearrange("p (n q) -> p n q", q=P),
                        AF.Identity)
                    if n < len(grp):
                        jb = grp[n]
                        w = J - 5 * P
                        nc.vector.tensor_copy(
                            ut_s[:w, jb, bc * P:bc * P + P],
                            pst[:w, n * P:(n + 1) * P])
                # alternate psum pools so the four transpose groups pipeline
                tgroup((0, 1, 2), ptr, "ptr")
                tgroup((3, 4, 5), pfe, f"pfe{bc}")

            def fe_all():
                # rotate through three psum slots (pdot is idle after exps)
                for o in range(CHB):
                    if o % 3 < 2:
                        acc = pfe.tile([P, BL], F32, tag=f"pfe{o % 3}")
                    else:
                        acc = pdot.tile([P, BL], F32, tag="pdot")
                    for t in range(3):
                        nc.tensor.matmul(acc[:], a8_s[:, 2 * t:2 * t + 2,
                                                      o * P:(o + 1) * P],
                                         ut_s[:, 2 * t:2 * t + 2, :],
                                         start=(t == 0), stop=(t == 2),
                                         perf_mode=DR)
                    dst = hfe_s[:, o, :]
                    if o % 2 == 0:
                        nc.scalar.activation(dst, acc[:], AF.Relu,
                                             scale=1.0 / (SA * SU))
                    else:
                        nc.vector.tensor_scalar(dst, acc[:],
                                                1.0 / (SA * SU), 0.0,
                                                op0=mybir.AluOpType.mult,
                                                op1=mybir.AluOpType.max)

            # ---- main interleave: the whole dots/softmax/transpose/fE chain
            # runs before the wv stream thickens; v blocks then track DMA ----
            dots(0)
            softmax_chain(0, nc.vector)
            dots(1)
            softmax_chain(1, nc.vector)
            v_block(0)
            v_block(1)
            out_mm(hv_s, 0)
            transpose_u(0)
            transpose_u(1)
            fe_all()
            for i in range(CHB):
                out_mm(hfe_s, i)
            for o in range(2, CHB):
                v_block(o)
                out_mm(hv_s, o - 1)
            out_mm(hv_s, CHB - 1)

            # ---- +bout, DMA out ----
            nc.vector.tensor_scalar_add(out_sb[:], pso[:], bout_s[:, 0:1])
            nc.sync.dma_start(out_e.ap(), out_sb[:])

    nc.compile()
    return nc


def host_prep(x, static_feat, Wk, bk, Wv, bv, WEk, bEk, WEv, bEv, Ww, bw,
              Wout, bout):
    """Host-side fp32 precompute, fp8/bf16 quantization, per-core input maps."""
    EPS = 1e-8
    f32 = np.float32
    x = np.asarray(x, f32)
    static_feat = np.asarray(static_feat, f32)
    Wk, bk = np.asarray(Wk, f32), np.asarray(bk, f32)
    Wv, bv = np.asarray(Wv, f32), np.asarray(bv, f32)
    Wout, bout = np.asarray(Wout, f32), np.asarray(bout, f32)

    Ek = np.einsum('oc,ncm->nom', np.asarray(WEk, f32), static_feat,
                   optimize=True) + np.asarray(bEk, f32)[None, :, None]
    Ev = np.einsum('oc,ncm->nom', np.asarray(WEv, f32), static_feat,
                   optimize=True) + np.asarray(bEv, f32)[None, :, None]
    Ekn = Ek / np.maximum(np.linalg.norm(Ek, axis=1, keepdims=True), EPS)
    Ekn_mat = Ekn.transpose(1, 0, 2).reshape(CH, J)          # [CH, 672]
    A_mat = Ev.transpose(0, 2, 1).reshape(J, CH)             # [672, CH]
    evwb = np.einsum('nom,o->nm', Ev, np.asarray(Ww, f32)[0]).reshape(J)

    # norm sketch + folded dots
    S = np.random.RandomState(0).randn(RSK, CH).astype(f32) / np.sqrt(RSK)
    W_sk = S @ Wk                                            # [128, CIN]
    b_sk = S @ bk
    D = Wk.T @ Ekn_mat                                       # [CIN, J]
    bkE = bk @ Ekn_mat                                       # [J]

    def cinlayout(w, width):    # [CIN, width] -> [P, KB*width]
        return np.ascontiguousarray(
            w.reshape(KB, P, width).transpose(1, 0, 2).reshape(P, KB * width))

    wsk8_h = cinlayout((W_sk.T * SD).astype(f8), RSK)
    d8_h = cinlayout((D * SD).astype(f8), J)

    def wlayout(w):     # [CIN, OCH] f8 -> dram [OCH/P, P, KB*P]
        och = w.shape[1]
        return np.ascontiguousarray(
            w.reshape(KB, P, och // P, P).transpose(2, 1, 0, 3)
            .reshape(och // P, P, KB * P))

    wv_s = Wv.T * SW
    wv8_f = wv_s.astype(f8)
    wv8lo_f = (wv_s - wv8_f.astype(f32)).astype(f8)
    wv8_h = wlayout(wv8_f)
    wv8lo_h = wlayout(wv8lo_f)

    a_pad = np.zeros((6 * P, CH), f32)
    a_pad[:J] = A_mat * SA
    a8_h = np.ascontiguousarray(
        a_pad.astype(f8).reshape(6, P, CH).transpose(1, 0, 2).reshape(P, 6 * CH))
    evwb_h = np.ascontiguousarray(
        np.broadcast_to(evwb.astype(bf16)[None, :], (P, J)))
    bke_h = (bkE * SD).astype(bf16).reshape(1, J)
    wout_sc = Wout.T.reshape(KB, P, NCLS).copy()
    for o in range(1, CHB, 2):       # hv blocks evicted via DVE carry x32
        wout_sc[o] /= SW
    wout_h = np.ascontiguousarray(
        wout_sc.transpose(1, 0, 2).reshape(P, KB * NCLS).astype(bf16))
    bsk_h = np.zeros((P, 1), f32)
    bsk_h[:RSK, 0] = b_sk * SD
    bv_h = np.ascontiguousarray(
        np.concatenate([bv.reshape(CHB, P).T, (bv * SW).reshape(CHB, P).T], 1))
    bout_h = bout.reshape(NCLS, 1)
    ident_h = np.eye(P, dtype=bf16)

    xT = np.ascontiguousarray(x[:, -1, :].T)                 # [CIN, B]
    x8_f = xT.astype(f8)
    x8lo_f = (xT - x8_f.astype(f32)).astype(f8)

    shared = dict(wsk8=wsk8_h, d8=d8_h, wv8=wv8_h, wv8lo=wv8lo_h, a8=a8_h,
                  evwb=evwb_h, bke=bke_h, wout=wout_h, bsk=bsk_h, bv=bv_h,
                  bout=bout_h, ident=ident_h)
    in_maps = []
    for c in range(NCORES):
        sl = slice(c * BL, (c + 1) * BL)

        def xlayout(xf):
            return np.ascontiguousarray(
                xf[:, sl].reshape(KB, P, BL).transpose(1, 0, 2)
                .reshape(P, KB * BL))
        in_maps.append(dict(x8=xlayout(x8_f), x8lo=xlayout(x8lo_f), **shared))
    return in_maps


_NC_CACHE = {}


def get_nc(debug=False):
    if debug not in _NC_CACHE:
        _NC_CACHE[debug] = build_nc(debug=debug)
    return _NC_CACHE[debug]


def kernel(**inputs) -> np.ndarray:
    nc = get_nc()
    in_maps = host_prep(**inputs)
    res = run_bass_kernel_spmd(nc, in_maps, list(range(NCORES)))
    out = np.empty((B, NCLS, 1), dtype=np.float32)
    for c in range(NCORES):
        out[c * BL:(c + 1) * BL, :, 0] = res.results[c]["out"].T
    return out
